# revision 58
# baseline (speedup 1.0000x reference)
"""Multi-head causal attention (B=4, S=2048, D=1024, H=16) on 8 NeuronCores.

Sharding: core c handles batch b = c//2 and head-group g = c%2 (8 heads).
Each core computes QKV projections for its group, causal attention for its
8 heads, and a partial output projection (row-split Wo).  Host sums the two
fp16 partials per batch in fp32 and adds bo.

On-chip design (per core):
  XT [D, S] = X[b].T fp16 in 8 chunks [128, S].
  Q/K are projected (stationary W-chunk x moving XT) and evacuated from
  PSUM with a fused bias-add straight to fp8e4 tiles qt8/kt8 [128, 2, S]:
  partition p of tile-group tg holds head 4*tg + p//32, dim1 selects the
  hd half (hd = dim1*32 + p%32).  Host permutes Wq/Wk columns to produce
  this layout directly.
  Scores run as fp8 DoubleRow matmuls (2 k-tiles of 32 partitions), giving
  qk [128 k, 2 heads, 512 q] PSUM blocks at half the fp16 cycle cost; exp
  on ScalarE (scale=1/8, no max subtraction; |scores| <= ~3) into fp16 et
  tiles (tag sets alternate by pair parity so the next pair's exps overlap
  the previous pair's PV reads; window 0 gets one set per pair).  Causal
  masking of diagonal 128x128 blocks happens inside the scores PSUM
  accumulation: an identity-stationary matmul adds 0/-30000 so exp zeroes
  the upper triangle with no extra vector pass.
  PV is TRANSPOSED: po [q=128, 65] += et_t[:, hh, qcol].T @ v_t[:, h, 0:65]
  accumulated over t (col 64 of v is ones and yields the exp rowsum), so
  the softmax divisor lands in a PSUM column: DVE reciprocal of po[:,64]
  then a per-partition tensor_scalar multiply evacuates the normalized
  [q, 64] tile in one shot (no scatter/gather/broadcast DMAs).  PV blocks
  emit all chains+norms first and the PE transposes (identity matmul,
  fp16 PSUM, two heads via tile_position col 0/64, two q-chunks per tile)
  afterwards, keeping DVE round-trip waits off the PE queue.
  O-projection runs at st-chunk granularity, evacuated fp32->fp16 on DVE
  (ScalarE Copy for the tail chunks), stored via per-half [128, 512] fp16
  DMAs; each window's O-projection is deferred two windows so the PE work
  lands in ACT-bound stretches.
  Emission is j-window-outer / pair-inner with V-projection chunks and
  Q/K projection groups interleaved as PE fillers, window 1's first score
  blocks pulled into window 0's tail, and each next window's first score
  block pulled into the previous weave, so ScalarE stays fed.

Walrus wait-slot legality (1 sem wait per engine instruction): Tile's
extra waits are split onto same-engine NoOps by _legalize_waits.
"""

import sys

for _p in ("/opt/trn_rl_repo",):
    if _p not in sys.path:
        sys.path.insert(0, _p)

from contextlib import ExitStack

import numpy as np

import concourse.bass as bass
import concourse.mybir as mybir
import concourse.tile as tile
from concourse.bass_utils import run_bass_kernel_spmd

import bass_rust

F16 = mybir.dt.float16
F32 = mybir.dt.float32
F8 = mybir.dt.float8e4
AF = mybir.ActivationFunctionType
DR = mybir.MatmulPerfMode.DoubleRow

B, S, D, H = 4, 2048, 1024, 16
HD = D // H  # 64
GH = 8  # heads per group
GW = GH * HD  # 512 columns per group


_SPLITTABLE = {
    "InstMatmult", "InstLdweights", "InstActivation", "InstTensorCopy",
    "InstTensorTensor", "InstTensorScalarPtr", "InstTensorReduce",
    "InstMemset", "InstDMACopy", "InstReciprocal", "InstIota",
    "InstTensorTensorReduce", "InstBNStats", "InstBNStatsAggregate",
    "InstStreamShuffle", "InstNoOp", "InstPool", "InstMax", "InstDrain",
}


def _legalize_waits(nc, max_waits=1):
    """Walrus codegen accepts at most one sync-wait command per engine
    instruction; Tile's wait assigner can emit more.  Split extras onto
    same-engine NoOps inserted immediately before (semantics preserved:
    the engine blocks at the same program point)."""
    ctr = 0
    for fn in nc.m.functions:
        for blk in fn.blocks:
            out = []
            for ins in blk.instructions:
                si = ins.sync_info
                if (
                    si is not None
                    and len(si.on_wait) > max_waits
                    and type(ins).__name__ in _SPLITTABLE
                ):
                    waits = list(si.on_wait)
                    extra, keep = waits[:-max_waits], waits[-max_waits:]
                    for w in extra:
                        nop = mybir.InstNoOp(name=f"waitnop-{ctr}", ins=[], outs=[])
                        ctr += 1
                        nop.engine = ins.engine
                        nop.sync_info = bass_rust.SyncInfo(on_wait=[w], on_update=[])
                        out.append(nop)
                    ins.sync_info = bass_rust.SyncInfo(
                        on_wait=keep, on_update=list(si.on_update)
                    )
                out.append(ins)
            blk.instructions[:] = out
    return ctr


def build_nc(s=S, fp8_proj=False, legalize=True):
    ns = s // 512  # 512-wide q windows
    nt = s // 128  # 128-wide s chunks
    nd = D // 128  # contraction chunks for projections

    nc = bass.Bass("TRN2", target_bir_lowering=False, debug=False)
    xt_d = nc.dram_tensor("xt", [D, s], F16, kind="ExternalInput").ap()
    if fp8_proj:
        xt8_d = nc.dram_tensor("xt8", [128, nd, s], F8, kind="ExternalInput").ap()
        wq_d = nc.dram_tensor("wq", [128, nd // 2, 2, GW], F8, kind="ExternalInput").ap()
        wk_d = nc.dram_tensor("wk", [128, nd // 2, 2, GW], F8, kind="ExternalInput").ap()
    else:
        wq_d = nc.dram_tensor("wq", [D, GW], F16, kind="ExternalInput").ap()
        wk_d = nc.dram_tensor("wk", [D, GW], F16, kind="ExternalInput").ap()
    wv_d = nc.dram_tensor("wv", [D, GW], F16, kind="ExternalInput").ap()
    wo_d = nc.dram_tensor("wo", [GW, D], F16, kind="ExternalInput").ap()
    bqk_d = nc.dram_tensor("bqk", [128, 8], F32, kind="ExternalInput").ap()
    bvb_d = nc.dram_tensor("bvb", [128, GW], F16, kind="ExternalInput").ap()
    mask_d = nc.dram_tensor("mask", [128, 128], F16, kind="ExternalInput").ap()
    iden_d = nc.dram_tensor("iden", [128, 128], F16, kind="ExternalInput").ap()
    out_d = nc.dram_tensor("out", [s, D], F16, kind="ExternalOutput").ap()

    with tile.TileContext(nc) as tc, ExitStack() as ctx:
        pool = lambda name, bufs, **kw: ctx.enter_context(
            tc.tile_pool(name=name, bufs=bufs, **kw)
        )
        const_p = pool("const", 1)
        xt_p = pool("xtp", nd)
        w_p = pool("wp", 1)
        qkt_p = pool("qktp", 1)
        v_p = pool("vp", nt)
        et_p = pool("etp", 1)  # tags e{parity}_{t}, one buf each
        ot_p = pool("otp", 4)
        on_p = pool("onp", 3)
        rec_p = pool("recp", 3)
        osb_p = pool("osbp", 3)
        ps_qk = pool("psqk", 2, space="PSUM")      # [128,2,512] f32 -> 4 banks
        ps_main = pool("psmain", 3, space="PSUM")  # [128,512] f32 -> 3 banks
        ps_tr = pool("pstr", 1, space="PSUM")      # [128,128] f16 -> 1 bank

        # --- input DMAs, in order of first use ---
        bqk_sb = const_p.tile([128, 8], F32)
        nc.sync.dma_start(out=bqk_sb[:], in_=bqk_d[:])
        if fp8_proj:
            xt8_sb = w_p.tile([128, nd, s], F8, name="xt8")
            nc.sync.dma_start(out=xt8_sb[:, 0:2, :], in_=xt8_d[:, 0:2, :])
            wq_sb = w_p.tile([128, nd // 2, 2, GW], F8, name="wq8")
            wk_sb = w_p.tile([128, nd // 2, 2, GW], F8, name="wk8")
            nc.sync.dma_start(out=wq_sb[:], in_=wq_d[:])
            nc.sync.dma_start(out=wk_sb[:], in_=wk_d[:])
            for dd in range(1, nd // 2):
                nc.sync.dma_start(
                    out=xt8_sb[:, 2 * dd : 2 * dd + 2, :],
                    in_=xt8_d[:, 2 * dd : 2 * dd + 2, :],
                )
            mask_sb = const_p.tile([128, 128], F16)
            nc.sync.dma_start(out=mask_sb[:], in_=mask_d[:])
            iden_sb = const_p.tile([128, 128], F16)
            nc.sync.dma_start(out=iden_sb[:], in_=iden_d[:])
            bvb_sb = const_p.tile([128, GW], F16)
            nc.sync.dma_start(out=bvb_sb[:], in_=bvb_d[:])
            wv_sb = w_p.tile([128, nd, GW], F16, name="wv")
            nc.sync.dma_start(out=wv_sb[:], in_=wv_d.rearrange("(d p) n -> p d n", p=128))
            xt_sb = []
            for d in range(nd):
                t = xt_p.tile([128, s], F16, tag="xt", name=f"xtc{d}")
                nc.sync.dma_start(out=t[:], in_=xt_d[d * 128 : (d + 1) * 128, :])
                xt_sb.append(t)
        else:
            wq_sb = w_p.tile([128, nd, GW], F16, name="wq")
            nc.sync.dma_start(out=wq_sb[:], in_=wq_d.rearrange("(d p) n -> p d n", p=128))
            xt_sb = []
            for d in range(nd):
                t = xt_p.tile([128, s], F16, tag="xt", name=f"xtc{d}")
                nc.sync.dma_start(out=t[:], in_=xt_d[d * 128 : (d + 1) * 128, :])
                xt_sb.append(t)
            wk_sb = w_p.tile([128, nd, GW], F16, name="wk")
            nc.sync.dma_start(out=wk_sb[:], in_=wk_d.rearrange("(d p) n -> p d n", p=128))
            bvb_sb = const_p.tile([128, GW], F16)
            nc.sync.dma_start(out=bvb_sb[:], in_=bvb_d[:])
            mask_sb = const_p.tile([128, 128], F16)
            nc.sync.dma_start(out=mask_sb[:], in_=mask_d[:])
            wv_sb = w_p.tile([128, nd, GW], F16, name="wv")
            nc.sync.dma_start(out=wv_sb[:], in_=wv_d.rearrange("(d p) n -> p d n", p=128))
        if not fp8_proj:
            iden_sb = const_p.tile([128, 128], F16)
            nc.sync.dma_start(out=iden_sb[:], in_=iden_d[:])
        wo_sb = w_p.tile([128, 4, D], F16, name="wo")
        nc.sync.dma_start(out=wo_sb[:], in_=wo_d.rearrange("(c p) n -> p c n", p=128))

        # touch ops: early Exp ACT-table load + const observations
        scr_a = const_p.tile([128, 1], F32)
        nc.scalar.activation(scr_a[:], bqk_sb[:, 0:1], AF.Exp, scale=1.0)
        scr_v = const_p.tile([128, 1], F16)
        nc.vector.tensor_copy(scr_v[:], bvb_sb[:, 0:1])
        scr_m = const_p.tile([128, 1], F16)
        nc.vector.tensor_copy(scr_m[:], mask_sb[:, 0:1])
        scr_i = const_p.tile([128, 1], F16)
        nc.gpsimd.tensor_copy(scr_i[:], iden_sb[:, 0:1])

        ot_sb = [ot_p.tile([128, s], F16, tag="ot", name=f"ot{m}") for m in range(4)]
        qt8 = [qkt_p.tile([128, 2, s], F8, name=f"qt8_{tg}") for tg in range(2)]
        kt8 = [qkt_p.tile([128, 2, s], F8, name=f"kt8_{tg}") for tg in range(2)]

        # --- projection-group emitters (used as PE fillers) ---
        def qk_group(pi, tg, inst, sl):
            """One Q/K projection PSUM group -> fp8 evacuation w/ bias."""
            dst = (qt8, kt8)[pi][tg]
            col = (tg * 2 + inst) * 128
            ps = ps_main.tile([128, 512], F32, tag="ps", name="ps")
            if fp8_proj:
                for dd in range(nd // 2):
                    nc.tensor.matmul(
                        ps[:],
                        wq_sb[:, dd, :, col : col + 128] if pi == 0
                        else wk_sb[:, dd, :, col : col + 128],
                        xt8_sb[:, 2 * dd : 2 * dd + 2, sl * 512 : (sl + 1) * 512],
                        start=(dd == 0),
                        stop=(dd == nd // 2 - 1),
                        perf_mode=DR,
                    )
            else:
                for d in range(nd):
                    nc.tensor.matmul(
                        ps[:],
                        (wq_sb, wk_sb)[pi][:, d, col : col + 128],
                        xt_sb[d][:, sl * 512 : (sl + 1) * 512],
                        start=(d == 0),
                        stop=(d == nd - 1),
                    )
            bc = pi * 4 + tg * 2 + inst
            nc.vector.tensor_scalar_add(
                dst[:, inst, sl * 512 : (sl + 1) * 512], ps[:], bqk_sb[:, bc : bc + 1]
            )

        v_sb = [None] * nt

        def v_group_multi(sts):
            """d-major interleave across <=3 chunks so only the last
            d-matmul of each chain waits the final xt DMA."""
            pss = []
            for st in sts:
                pss.append(ps_main.tile([128, 512], F32, tag="ps", name="ps"))
            for d in range(nd):
                for ps, st in zip(pss, sts):
                    nc.tensor.matmul(
                        ps[:],
                        xt_sb[d][:, st * 128 : (st + 1) * 128],
                        wv_sb[:, d, :],
                        start=(d == 0),
                        stop=(d == nd - 1),
                    )
            for ps, st in zip(pss, sts):
                vt = v_p.tile([128, GH, 65], F16, tag="v", name=f"v{st}")
                nc.vector.memset(vt[:, :, 64:65], 1.0)
                nc.vector.tensor_add(
                    vt[:, :, 0:64],
                    ps[:].rearrange("p (h e) -> p h e", h=GH),
                    bvb_sb[:].rearrange("p (h e) -> p h e", h=GH),
                )
                v_sb[st] = vt

        def v_group(st):
            """One V-projection s-chunk [128, 8, 65] with ones col."""
            ps = ps_main.tile([128, 512], F32, tag="ps", name="ps")
            for d in range(nd):
                nc.tensor.matmul(
                    ps[:],
                    xt_sb[d][:, st * 128 : (st + 1) * 128],
                    wv_sb[:, d, :],
                    start=(d == 0),
                    stop=(d == nd - 1),
                )
            vt = v_p.tile([128, GH, 65], F16, tag="v", name=f"v{st}")
            nc.vector.memset(vt[:, :, 64:65], 1.0)
            nc.vector.tensor_add(
                vt[:, :, 0:64],
                ps[:].rearrange("p (h e) -> p h e", h=GH),
                bvb_sb[:].rearrange("p (h e) -> p h e", h=GH),
            )
            v_sb[st] = vt

        # --- attention building blocks ---
        et_tiles = [[None] * nt for _ in range(4)]  # by pair parity (j0: per pair)

        def scores_t(m, j, t):
            tg, i0 = m // 2, 2 * (m % 2)
            par = m if j == 0 else m % 2
            diag = t >= 4 * j
            w0 = 128 * (t - 4 * j) if diag else 0
            qk = ps_qk.tile([128, 2, 512], F32, tag="qk", name="qk")
            for hh in range(2):
                ib = 32 * (i0 + hh)
                nc.tensor.matmul(
                    qk[:, hh, w0:512],
                    kt8[tg][ib : ib + 32, :, t * 128 : (t + 1) * 128],
                    qt8[tg][ib : ib + 32, :, j * 512 + w0 : (j + 1) * 512],
                    start=True,
                    stop=not diag,
                    perf_mode=DR,
                    tile_position=(ib, 0),
                    skip_group_check=diag,
                )
                if diag:
                    # causal mask: accumulate 0/-30000 into the diagonal
                    # 128x128 block (identity-stationary matmul) so exp
                    # zeroes the upper triangle with no DVE pass
                    nc.tensor.matmul(
                        qk[:, hh, w0 : w0 + 128],
                        iden_sb[:],
                        mask_sb[:],
                        start=False,
                        stop=True,
                        skip_group_check=True,
                    )
            et = et_p.tile([128, 2, 512], F16, tag=f"e{par}_{t}", name=f"e{par}_{t}")
            nc.scalar.activation(
                et[:, :, w0:512], qk[:, :, w0:512], AF.Exp, scale=0.125
            )
            et_tiles[par][t] = et

        def scores_block(m, j):
            for t in range(4 * j + 4):
                scores_t(m, j, t)

        trp_cur = {}

        def pv_chains(m, j, qq):
            """PV chains + norms only (no PE-blocking transpose waits)."""
            tg, i0 = m // 2, 2 * (m % 2)
            par = m if j == 0 else m % 2
            h0, h1 = 4 * tg + i0, 4 * tg + i0 + 1
            qc = 4 * j + qq
            pair = []
            for hh, h in ((0, h0), (1, h1)):
                po = ps_main.tile([128, 512], F32, tag="ps", name="po")
                for t in range(qc + 1):
                    nc.tensor.matmul(
                        po[:, 0:65],
                        et_tiles[par][t][:, hh, qq * 128 : (qq + 1) * 128],
                        v_sb[t][:, h, 0:65],
                        start=(t == 0),
                        stop=(t == qc),
                    )
                rec = rec_p.tile([128, 1], F32, tag="rec", name="rec")
                nc.vector.reciprocal(rec[:], po[:, 64:65])
                on = on_p.tile([128, 64], F16, tag=f"dn{qq}_{hh}", name="dn", bufs=1)
                nc.vector.tensor_scalar_mul(on[:], po[:, 0:64], rec[:, 0:1])
                pair.append(on)
            return pair

        def pv_flush(m, j, ons):
            for qp in range(len(ons) // 2):
                trp = ps_tr.tile([128, 2, 128], F16, tag="tr", name="trp")
                for sub in range(2):
                    for hh in range(2):
                        nc.tensor.transpose(
                            trp[64 * hh : 64 * hh + 64, sub, :],
                            ons[2 * qp + sub][hh][:],
                            iden_sb[:],
                            tile_position=(0, 64 * hh),
                        )
                qc = 4 * j + 2 * qp + 1
                nc.vector.tensor_copy(
                    ot_sb[m][:, (qc - 1) * 128 : (qc + 1) * 128],
                    trp[:].rearrange("p a b -> p (a b)"),
                )

        def pv_chunk_defer(m, j):
            ons = [pv_chains(m, j, qq) for qq in range(4)]
            return lambda: pv_flush(m, j, ons)

        def pv_chunk(m, j, qq):
            tg, i0 = m // 2, 2 * (m % 2)
            par = m if j == 0 else m % 2
            h0, h1 = 4 * tg + i0, 4 * tg + i0 + 1
            qc = 4 * j + qq
            # transpose targets pair up (two qc's per [128, 256] fp16 PSUM
            # tile) so the DVE evacuation is one copy per pair; the last
            # pair's O-proj chunks stay per-qc for the tail
            solo = m == 3 and j == ns - 1 and qq >= 2  # tail chunks stream per-qc
            if qq % 2 == 0 or solo:
                trp_cur[m] = ps_tr.tile([128, 2, 128], F16, tag="tr", name="trp")
            trp = trp_cur[m]
            for hh, h in ((0, h0), (1, h1)):
                po = ps_main.tile([128, 512], F32, tag="ps", name="po")
                for t in range(qc + 1):
                    nc.tensor.matmul(
                        po[:, 0:65],
                        et_tiles[par][t][:, hh, qq * 128 : (qq + 1) * 128],
                        v_sb[t][:, h, 0:65],
                        start=(t == 0),
                        stop=(t == qc),
                    )
                rec = rec_p.tile([128, 1], F32, tag="rec", name="rec")
                nc.vector.reciprocal(rec[:], po[:, 64:65])
                on = on_p.tile([128, 64], F16, tag="on", name="on")
                nc.vector.tensor_scalar_mul(on[:], po[:, 0:64], rec[:, 0:1])
                nc.tensor.transpose(
                    trp[64 * hh : 64 * hh + 64, 0 if solo else qq % 2, :],
                    on[:],
                    iden_sb[:],
                    tile_position=(0, 64 * hh),
                )
            if solo:
                nc.vector.tensor_copy(
                    ot_sb[m][:, qc * 128 : (qc + 1) * 128], trp[:, 0, :]
                )
                oproj_chunk(j, qq, act_evac=True)
            elif qq % 2 == 1:
                nc.vector.tensor_copy(
                    ot_sb[m][:, (qc - 1) * 128 : (qc + 1) * 128],
                    trp[:].rearrange("p a b -> p (a b)"),
                )
                if m == 3 and j == ns - 1:
                    oproj_chunk(j, qq - 1)
                    oproj_chunk(j, qq)

        def pv_block(m, j):
            pv_chunk_defer(m, j)()

        def scores_pv_block(m, j, mid=None, post=None):
            """Scores with the PV chunks woven between the diagonal t's so
            the window tail overlaps PE work with the last exps.  `post`
            (next window's first scores) lands after the first PV chunk."""
            for t in range(4 * j + 1):
                scores_t(m, j, t)
            if mid is not None:
                mid()
            if j == ns - 1:
                # tail: inline transposes/oproj so output chunks stream
                for qq in range(4):
                    if qq < 3:
                        scores_t(m, j, 4 * j + 1 + qq)
                    pv_chunk(m, j, qq)
                    if qq == 2 and post is not None:
                        post()
            else:
                ons = []
                for qq in range(4):
                    if qq < 3:
                        scores_t(m, j, 4 * j + 1 + qq)
                    ons.append(pv_chains(m, j, qq))
                    if qq == 2 and post is not None:
                        post()
                pv_flush(m, j, ons)

        def oproj_chunk(j, qq, act_evac=False):
            st = 4 * j + qq
            for dsl in range(2):
                po2 = ps_main.tile([128, 512], F32, tag="ps", name="po2")
                for cc in range(4):
                    nc.tensor.matmul(
                        po2[:],
                        ot_sb[cc][:, st * 128 : (st + 1) * 128],
                        wo_sb[:, cc, dsl * 512 : (dsl + 1) * 512],
                        start=(cc == 0),
                        stop=(cc == 3),
                    )
                osb = osb_p.tile([128, 512], F16, tag="osb", name="osb")
                if act_evac and dsl == 1:
                    # post-exp tail: ScalarE is idle, split the serial
                    # evacuation chain across both engines
                    nc.scalar.activation(osb[:], po2[:], AF.Copy)
                else:
                    nc.vector.tensor_copy(osb[:], po2[:])
                nc.sync.dma_start(
                    out=out_d[
                        st * 128 : (st + 1) * 128, dsl * 512 : (dsl + 1) * 512
                    ],
                    in_=osb[:],
                )

        # --- emission: prologue + j-outer / pair-inner windows ---
        # Projection-group units: (tg, sl) -> 4 groups (Q i0/i1, K i0/i1).
        # Emitted lazily as PE fillers; ensure() forces a unit's deadline.
        unit_q = [(0, 0)] + [
            (tg, sl) for sl in range(ns) for tg in range(2) if (tg, sl) != (0, 0)
        ]
        done_units = set()

        def emit_unit(tg, sl):
            for pi in range(2):
                for inst in range(2):
                    qk_group(pi, tg, inst, sl)
            done_units.add((tg, sl))

        def ensure(tg, sl):
            while (tg, sl) not in done_units:
                fill_unit()

        def fill_unit():
            for u in unit_q:
                if u not in done_units:
                    emit_unit(*u)
                    return

        fill_unit()  # (0, 0) prologue

        def oproj_block(j):
            for qq in range(4):
                oproj_chunk(j, qq)

        # Window 0: V (fp16 xt) lands late in the DMA stream and the window
        # has little exp work, so all four pairs' scores are front-loaded,
        # V chunks woven into pair 3's diagonal, and window 1's first two
        # score blocks pulled in to cover the PV tail.  O-projections are
        # deferred two windows (PE work moves into the ACT-bound windows).
        scores_block(0, 0)
        scores_block(1, 0)
        ensure(1, 0)
        if ns > 1:
            ensure(0, 1)
        fill_unit()  # (1,1): unblocks window 1 while PE/DVE are idle
        scores_block(2, 0)
        scores_t(3, 0, 0)
        scores_t(3, 0, 1)
        scores_t(3, 0, 2)
        scores_t(3, 0, 3)
        v_group_multi([0, 1, 2])
        v_group(3)
        if ns > 1:
            flush0 = pv_chunk_defer(0, 0)
            scores_block(0, 1)
            flush0()
        else:
            pv_block(0, 0)
        pv_block(1, 0)
        if ns > 1:
            scores_block(1, 1)
            pv_block(2, 0)
            v_group(4)
            v_group(5)
        else:
            pv_block(2, 0)
            for qq in range(4):
                pv_chunk(3, 0, qq)

        for j in range(1, ns):
            if j > 1:
                v_group(4 * j)  # scores_block(0, j) was pulled into the
                scores_block(1, j)  # previous window's weave
                v_group(4 * j + 1)
            ensure(1, j)
            v_group(4 * j + 2)
            v_group(4 * j + 3)
            pv_block(0, j)
            if j >= 2:
                oproj_block(j - 2)
            scores_block(2, j)
            if j == ns - 1 and j >= 1:
                oproj_block(j - 1)
            pv_block(1, j)
            if j == 1:
                pv_block(3, 0)
            last = j == ns - 1

            def mid(j=j):
                pv_block(2, j)

            def post(j=j, last=last):
                if not last:
                    ensure(0, j + 1)
                    scores_block(0, j + 1)

            scores_pv_block(3, j, mid=mid, post=post)

    if legalize:
        _legalize_waits(nc)
    return nc


_NC_CACHE = {}
FP8_PROJ = True


def _get_nc(s=S):
    key = (s, FP8_PROJ)
    if key not in _NC_CACHE:
        _NC_CACHE[key] = build_nc(s, fp8_proj=FP8_PROJ)
    return _NC_CACHE[key]


def _perm(lo):
    """Column permutation for Wq/Wk: new col a = tg*256 + inst*128 + p maps
    to original output dim lo + 64*(4*tg + p//32) + inst*32 + p%32."""
    a = np.arange(GW)
    tg, r = a // 256, a % 256
    inst, p = r // 128, r % 128
    return lo + 64 * (4 * tg + p // 32) + inst * 32 + (p % 32)


def _to_f8(a):
    import ml_dtypes

    return a.astype(ml_dtypes.float8_e4m3)


def make_inputs(X, Wq, bq, Wk, bk, Wv, bv, Wo, bo, s=S, fp8_proj=False):
    """Per-core input maps. Core c: batch c//2, head group c%2."""
    iv, jv = np.arange(128)[:, None], np.arange(128)[None, :]
    mask = np.where(jv >= iv, 0.0, -30000.0).astype(np.float16)
    iden = np.eye(128, dtype=np.float16)
    nd = D // 128
    in_maps = []
    for c in range(8):
        b, g = divmod(c, 2)
        lo, hi = g * GW, (g + 1) * GW
        perm = _perm(lo)
        bqk = np.empty((128, 8), np.float32)
        for pi, bias in enumerate((bq, bk)):
            for tg in range(2):
                for inst in range(2):
                    cols = perm[tg * 256 + inst * 128 : tg * 256 + inst * 128 + 128]
                    bqk[:, pi * 4 + tg * 2 + inst] = bias[cols]
        xt = np.ascontiguousarray(X[b, :s].T).astype(np.float16)
        m = {
            "xt": xt,
            "wv": np.ascontiguousarray(Wv[lo:hi].T).astype(np.float16),
            "wo": np.ascontiguousarray(Wo[:, lo:hi].T).astype(np.float16),
            "bqk": bqk,
            "bvb": np.tile(bv[lo:hi].astype(np.float16), (128, 1)),
            "mask": mask,
            "iden": iden,
        }
        if fp8_proj:
            # xt8 [128, nd, s]: [p, d, s] = X.T[d*128+p, s]
            m["xt8"] = _to_f8(xt.reshape(nd, 128, s).transpose(1, 0, 2))
            for nm, W in (("wq", Wq), ("wk", Wk)):
                wt = np.ascontiguousarray(W[perm].T)  # [D, GW]
                # [128, nd//2, 2, GW]: [p, dd, i, c] = wt[(2dd+i)*128+p, c]
                m[nm] = _to_f8(
                    wt.reshape(nd // 2, 2, 128, GW).transpose(2, 0, 1, 3)
                )
        else:
            m["wq"] = np.ascontiguousarray(Wq[perm].T).astype(np.float16)
            m["wk"] = np.ascontiguousarray(Wk[perm].T).astype(np.float16)
        in_maps.append(m)
    return in_maps


def kernel(X, Wq, bq, Wk, bk, Wv, bv, Wo, bo, **run_kwargs):
    args = [np.asarray(a, np.float32) for a in (X, Wq, bq, Wk, bk, Wv, bv, Wo, bo)]
    X, Wq, bq, Wk, bk, Wv, bv, Wo, bo = args
    nc = _get_nc(S)
    in_maps = make_inputs(X, Wq, bq, Wk, bk, Wv, bv, Wo, bo, S, fp8_proj=FP8_PROJ)
    res = run_bass_kernel_spmd(nc, in_maps, core_ids=list(range(8)), **run_kwargs)
    outs = [r["out"] for r in res.results]
    full = np.empty((B, S, D), np.float32)
    for b in range(B):
        full[b] = outs[2 * b].astype(np.float32) + outs[2 * b + 1].astype(np.float32) + bo
    kernel.last_results = res
    return full


# revision 63
# speedup vs baseline: 1.0035x; 1.0035x over previous
"""Multi-head causal attention (B=4, S=2048, D=1024, H=16) on 8 NeuronCores.

Sharding: core c handles batch b = c//2 and head-group g = c%2 (8 heads).
Each core computes QKV projections for its group, causal attention for its
8 heads, and a partial output projection (row-split Wo).  Host sums the two
fp16 partials per batch in fp32 and adds bo.

On-chip design (per core):
  XT [D, S] = X[b].T fp16 in 8 chunks [128, S].
  Q/K are projected (stationary W-chunk x moving XT) and evacuated from
  PSUM with a fused bias-add straight to fp8e4 tiles qt8/kt8 [128, 2, S]:
  partition p of tile-group tg holds head 4*tg + p//32, dim1 selects the
  hd half (hd = dim1*32 + p%32).  Host permutes Wq/Wk columns to produce
  this layout directly.
  Scores run as fp8 DoubleRow matmuls (2 k-tiles of 32 partitions), giving
  qk [128 k, 2 heads, 512 q] PSUM blocks at half the fp16 cycle cost; exp
  on ScalarE (scale=1/8, no max subtraction; |scores| <= ~3) into fp16 et
  tiles (tag sets alternate by pair parity so the next pair's exps overlap
  the previous pair's PV reads; window 0 gets one set per pair).  Causal
  masking of diagonal 128x128 blocks happens inside the scores PSUM
  accumulation: an identity-stationary matmul adds 0/-30000 so exp zeroes
  the upper triangle with no extra vector pass.
  PV is TRANSPOSED: po [q=128, 65] += et_t[:, hh, qcol].T @ v_t[:, h, 0:65]
  accumulated over t (col 64 of v is ones and yields the exp rowsum), so
  the softmax divisor lands in a PSUM column: DVE reciprocal of po[:,64]
  then a per-partition tensor_scalar multiply evacuates the normalized
  [q, 64] tile in one shot (no scatter/gather/broadcast DMAs).  PV blocks
  emit all chains+norms first and the PE transposes (identity matmul,
  fp16 PSUM, two heads via tile_position col 0/64, two q-chunks per tile)
  afterwards, keeping DVE round-trip waits off the PE queue.
  O-projection runs at st-chunk granularity, evacuated fp32->fp16 on DVE
  (ScalarE Copy for the tail chunks), stored via per-half [128, 512] fp16
  DMAs; each window's O-projection is deferred two windows so the PE work
  lands in ACT-bound stretches.
  Emission is j-window-outer / pair-inner with V-projection chunks and
  Q/K projection groups interleaved as PE fillers, window 1's first score
  blocks pulled into window 0's tail, and each next window's first score
  block pulled into the previous weave, so ScalarE stays fed.

Walrus wait-slot legality (1 sem wait per engine instruction): Tile's
extra waits are split onto same-engine NoOps by _legalize_waits.
"""

import sys

for _p in ("/opt/trn_rl_repo",):
    if _p not in sys.path:
        sys.path.insert(0, _p)

from contextlib import ExitStack

import numpy as np

import concourse.bass as bass
import concourse.mybir as mybir
import concourse.tile as tile
from concourse.bass_utils import run_bass_kernel_spmd

import bass_rust

F16 = mybir.dt.float16
F32 = mybir.dt.float32
F8 = mybir.dt.float8e4
AF = mybir.ActivationFunctionType
DR = mybir.MatmulPerfMode.DoubleRow

B, S, D, H = 4, 2048, 1024, 16
HD = D // H  # 64
GH = 8  # heads per group
GW = GH * HD  # 512 columns per group


_SPLITTABLE = {
    "InstMatmult", "InstLdweights", "InstActivation", "InstTensorCopy",
    "InstTensorTensor", "InstTensorScalarPtr", "InstTensorReduce",
    "InstMemset", "InstDMACopy", "InstReciprocal", "InstIota",
    "InstTensorTensorReduce", "InstBNStats", "InstBNStatsAggregate",
    "InstStreamShuffle", "InstNoOp", "InstPool", "InstMax", "InstDrain",
}


def _legalize_waits(nc, max_waits=1):
    """Walrus codegen accepts at most one sync-wait command per engine
    instruction; Tile's wait assigner can emit more.  Split extras onto
    same-engine NoOps inserted immediately before (semantics preserved:
    the engine blocks at the same program point)."""
    ctr = 0
    for fn in nc.m.functions:
        for blk in fn.blocks:
            out = []
            for ins in blk.instructions:
                si = ins.sync_info
                if (
                    si is not None
                    and len(si.on_wait) > max_waits
                    and type(ins).__name__ in _SPLITTABLE
                ):
                    waits = list(si.on_wait)
                    extra, keep = waits[:-max_waits], waits[-max_waits:]
                    for w in extra:
                        nop = mybir.InstNoOp(name=f"waitnop-{ctr}", ins=[], outs=[])
                        ctr += 1
                        nop.engine = ins.engine
                        nop.sync_info = bass_rust.SyncInfo(on_wait=[w], on_update=[])
                        out.append(nop)
                    ins.sync_info = bass_rust.SyncInfo(
                        on_wait=keep, on_update=list(si.on_update)
                    )
                out.append(ins)
            blk.instructions[:] = out
    return ctr


def build_nc(s=S, fp8_proj=False, legalize=True):
    ns = s // 512  # 512-wide q windows
    nt = s // 128  # 128-wide s chunks
    nd = D // 128  # contraction chunks for projections

    nc = bass.Bass("TRN2", target_bir_lowering=False, debug=False)
    xt_d = nc.dram_tensor("xt", [D, s], F16, kind="ExternalInput").ap()
    if fp8_proj:
        xt8_d = nc.dram_tensor("xt8", [128, nd, s], F8, kind="ExternalInput").ap()
        wq_d = nc.dram_tensor("wq", [128, nd // 2, 2, GW], F8, kind="ExternalInput").ap()
        wk_d = nc.dram_tensor("wk", [128, nd // 2, 2, GW], F8, kind="ExternalInput").ap()
    else:
        wq_d = nc.dram_tensor("wq", [D, GW], F16, kind="ExternalInput").ap()
        wk_d = nc.dram_tensor("wk", [D, GW], F16, kind="ExternalInput").ap()
    wv_d = nc.dram_tensor("wv", [D, GW], F16, kind="ExternalInput").ap()
    wo_d = nc.dram_tensor("wo", [GW, D], F16, kind="ExternalInput").ap()
    bqk_d = nc.dram_tensor("bqk", [128, 8], F32, kind="ExternalInput").ap()
    bvb_d = nc.dram_tensor("bvb", [128, GW], F16, kind="ExternalInput").ap()
    mask_d = nc.dram_tensor("mask", [128, 128], F16, kind="ExternalInput").ap()
    iden_d = nc.dram_tensor("iden", [128, 128], F16, kind="ExternalInput").ap()
    out_d = nc.dram_tensor("out", [s, D], F16, kind="ExternalOutput").ap()

    with tile.TileContext(nc) as tc, ExitStack() as ctx:
        pool = lambda name, bufs, **kw: ctx.enter_context(
            tc.tile_pool(name=name, bufs=bufs, **kw)
        )
        const_p = pool("const", 1)
        xt_p = pool("xtp", nd)
        w_p = pool("wp", 1)
        qkt_p = pool("qktp", 1)
        v_p = pool("vp", nt)
        et_p = pool("etp", 1)  # tags e{parity}_{t}, one buf each
        ot_p = pool("otp", 4)
        on_p = pool("onp", 3)
        rec_p = pool("recp", 3)
        osb_p = pool("osbp", 3)
        ps_qk = pool("psqk", 2, space="PSUM")      # [128,2,512] f32 -> 4 banks
        ps_main = pool("psmain", 3, space="PSUM")  # [128,512] f32 -> 3 banks
        ps_tr = pool("pstr", 1, space="PSUM")      # [128,128] f16 -> 1 bank

        # --- input DMAs, in order of first use ---
        bqk_sb = const_p.tile([128, 8], F32)
        nc.sync.dma_start(out=bqk_sb[:], in_=bqk_d[:])
        if fp8_proj:
            xt8_sb = w_p.tile([128, nd, s], F8, name="xt8")
            nc.sync.dma_start(out=xt8_sb[:, 0:2, :], in_=xt8_d[:, 0:2, :])
            wq_sb = w_p.tile([128, nd // 2, 2, GW], F8, name="wq8")
            wk_sb = w_p.tile([128, nd // 2, 2, GW], F8, name="wk8")
            nc.sync.dma_start(out=wq_sb[:], in_=wq_d[:])
            nc.sync.dma_start(out=wk_sb[:], in_=wk_d[:])
            for dd in range(1, nd // 2):
                nc.sync.dma_start(
                    out=xt8_sb[:, 2 * dd : 2 * dd + 2, :],
                    in_=xt8_d[:, 2 * dd : 2 * dd + 2, :],
                )
            mask_sb = const_p.tile([128, 128], F16)
            nc.sync.dma_start(out=mask_sb[:], in_=mask_d[:])
            iden_sb = const_p.tile([128, 128], F16)
            nc.sync.dma_start(out=iden_sb[:], in_=iden_d[:])
            bvb_sb = const_p.tile([128, GW], F16)
            nc.sync.dma_start(out=bvb_sb[:], in_=bvb_d[:])
            wv_sb = w_p.tile([128, nd, GW], F16, name="wv")
            nc.sync.dma_start(out=wv_sb[:], in_=wv_d.rearrange("(d p) n -> p d n", p=128))
            xt_sb = []
            for d in range(nd):
                t = xt_p.tile([128, s], F16, tag="xt", name=f"xtc{d}")
                nc.sync.dma_start(out=t[:], in_=xt_d[d * 128 : (d + 1) * 128, :])
                xt_sb.append(t)
        else:
            wq_sb = w_p.tile([128, nd, GW], F16, name="wq")
            nc.sync.dma_start(out=wq_sb[:], in_=wq_d.rearrange("(d p) n -> p d n", p=128))
            xt_sb = []
            for d in range(nd):
                t = xt_p.tile([128, s], F16, tag="xt", name=f"xtc{d}")
                nc.sync.dma_start(out=t[:], in_=xt_d[d * 128 : (d + 1) * 128, :])
                xt_sb.append(t)
            wk_sb = w_p.tile([128, nd, GW], F16, name="wk")
            nc.sync.dma_start(out=wk_sb[:], in_=wk_d.rearrange("(d p) n -> p d n", p=128))
            bvb_sb = const_p.tile([128, GW], F16)
            nc.sync.dma_start(out=bvb_sb[:], in_=bvb_d[:])
            mask_sb = const_p.tile([128, 128], F16)
            nc.sync.dma_start(out=mask_sb[:], in_=mask_d[:])
            wv_sb = w_p.tile([128, nd, GW], F16, name="wv")
            nc.sync.dma_start(out=wv_sb[:], in_=wv_d.rearrange("(d p) n -> p d n", p=128))
        if not fp8_proj:
            iden_sb = const_p.tile([128, 128], F16)
            nc.sync.dma_start(out=iden_sb[:], in_=iden_d[:])
        wo_sb = w_p.tile([128, 4, D], F16, name="wo")
        nc.sync.dma_start(out=wo_sb[:], in_=wo_d.rearrange("(c p) n -> p c n", p=128))

        # touch ops: early Exp ACT-table load + const observations
        scr_a = const_p.tile([128, 1], F32)
        nc.scalar.activation(scr_a[:], bqk_sb[:, 0:1], AF.Exp, scale=1.0)
        scr_v = const_p.tile([128, 1], F16)
        nc.vector.tensor_copy(scr_v[:], bvb_sb[:, 0:1])
        scr_m = const_p.tile([128, 1], F16)
        nc.vector.tensor_copy(scr_m[:], mask_sb[:, 0:1])
        scr_i = const_p.tile([128, 1], F16)
        nc.gpsimd.tensor_copy(scr_i[:], iden_sb[:, 0:1])

        ot_sb = [ot_p.tile([128, s], F16, tag="ot", name=f"ot{m}") for m in range(4)]
        qt8 = [qkt_p.tile([128, 2, s], F8, name=f"qt8_{tg}") for tg in range(2)]
        kt8 = [qkt_p.tile([128, 2, s], F8, name=f"kt8_{tg}") for tg in range(2)]

        # --- projection-group emitters (used as PE fillers) ---
        def qk_group(pi, tg, inst, sl, act_evac=False):
            """One Q/K projection PSUM group -> fp8 evacuation.  Q gets the
            bias folded in; K needs none (softmax is invariant to the key
            bias: the q.bk term is constant per query row and cancels in
            normalization), which also lets prologue K evacs run on the
            then-idle ScalarE."""
            dst = (qt8, kt8)[pi][tg]
            col = (tg * 2 + inst) * 128
            ps = ps_main.tile([128, 512], F32, tag="ps", name="ps")
            if fp8_proj:
                for dd in range(nd // 2):
                    nc.tensor.matmul(
                        ps[:],
                        wq_sb[:, dd, :, col : col + 128] if pi == 0
                        else wk_sb[:, dd, :, col : col + 128],
                        xt8_sb[:, 2 * dd : 2 * dd + 2, sl * 512 : (sl + 1) * 512],
                        start=(dd == 0),
                        stop=(dd == nd // 2 - 1),
                        perf_mode=DR,
                    )
            else:
                for d in range(nd):
                    nc.tensor.matmul(
                        ps[:],
                        (wq_sb, wk_sb)[pi][:, d, col : col + 128],
                        xt_sb[d][:, sl * 512 : (sl + 1) * 512],
                        start=(d == 0),
                        stop=(d == nd - 1),
                    )
            dslice = dst[:, inst, sl * 512 : (sl + 1) * 512]
            if pi == 1:
                if act_evac:
                    nc.scalar.activation(dslice, ps[:], AF.Copy)
                else:
                    nc.vector.tensor_copy(dslice, ps[:])
            else:
                bc = tg * 2 + inst
                nc.vector.tensor_scalar_add(dslice, ps[:], bqk_sb[:, bc : bc + 1])

        v_sb = [None] * nt

        def v_group_multi(sts):
            """d-major interleave across <=3 chunks so only the last
            d-matmul of each chain waits the final xt DMA."""
            pss = []
            for st in sts:
                pss.append(ps_main.tile([128, 512], F32, tag="ps", name="ps"))
            for d in range(nd):
                for ps, st in zip(pss, sts):
                    nc.tensor.matmul(
                        ps[:],
                        xt_sb[d][:, st * 128 : (st + 1) * 128],
                        wv_sb[:, d, :],
                        start=(d == 0),
                        stop=(d == nd - 1),
                    )
            for ps, st in zip(pss, sts):
                vt = v_p.tile([128, GH, 65], F16, tag="v", name=f"v{st}")
                nc.vector.memset(vt[:, :, 64:65], 1.0)
                nc.vector.tensor_add(
                    vt[:, :, 0:64],
                    ps[:].rearrange("p (h e) -> p h e", h=GH),
                    bvb_sb[:].rearrange("p (h e) -> p h e", h=GH),
                )
                v_sb[st] = vt

        def v_group(st):
            """One V-projection s-chunk [128, 8, 65] with ones col."""
            ps = ps_main.tile([128, 512], F32, tag="ps", name="ps")
            for d in range(nd):
                nc.tensor.matmul(
                    ps[:],
                    xt_sb[d][:, st * 128 : (st + 1) * 128],
                    wv_sb[:, d, :],
                    start=(d == 0),
                    stop=(d == nd - 1),
                )
            vt = v_p.tile([128, GH, 65], F16, tag="v", name=f"v{st}")
            nc.vector.memset(vt[:, :, 64:65], 1.0)
            nc.vector.tensor_add(
                vt[:, :, 0:64],
                ps[:].rearrange("p (h e) -> p h e", h=GH),
                bvb_sb[:].rearrange("p (h e) -> p h e", h=GH),
            )
            v_sb[st] = vt

        # --- attention building blocks ---
        et_tiles = [[None] * nt for _ in range(4)]  # by pair parity (j0: per pair)

        def scores_t(m, j, t):
            tg, i0 = m // 2, 2 * (m % 2)
            par = m if j == 0 else m % 2
            diag = t >= 4 * j
            w0 = 128 * (t - 4 * j) if diag else 0
            qk = ps_qk.tile([128, 2, 512], F32, tag="qk", name="qk")
            for hh in range(2):
                ib = 32 * (i0 + hh)
                nc.tensor.matmul(
                    qk[:, hh, w0:512],
                    kt8[tg][ib : ib + 32, :, t * 128 : (t + 1) * 128],
                    qt8[tg][ib : ib + 32, :, j * 512 + w0 : (j + 1) * 512],
                    start=True,
                    stop=not diag,
                    perf_mode=DR,
                    tile_position=(ib, 0),
                    skip_group_check=diag,
                )
                if diag:
                    # causal mask: accumulate 0/-30000 into the diagonal
                    # 128x128 block (identity-stationary matmul) so exp
                    # zeroes the upper triangle with no DVE pass
                    nc.tensor.matmul(
                        qk[:, hh, w0 : w0 + 128],
                        iden_sb[:],
                        mask_sb[:],
                        start=False,
                        stop=True,
                        skip_group_check=True,
                    )
            et = et_p.tile([128, 2, 512], F16, tag=f"e{par}_{t}", name=f"e{par}_{t}")
            nc.scalar.activation(
                et[:, :, w0:512], qk[:, :, w0:512], AF.Exp, scale=0.125
            )
            et_tiles[par][t] = et

        def scores_block(m, j):
            for t in range(4 * j + 4):
                scores_t(m, j, t)

        trp_cur = {}

        def pv_chains(m, j, qq):
            """PV chains + norms only (no PE-blocking transpose waits)."""
            tg, i0 = m // 2, 2 * (m % 2)
            par = m if j == 0 else m % 2
            h0, h1 = 4 * tg + i0, 4 * tg + i0 + 1
            qc = 4 * j + qq
            pair = []
            for hh, h in ((0, h0), (1, h1)):
                po = ps_main.tile([128, 512], F32, tag="ps", name="po")
                for t in range(qc + 1):
                    nc.tensor.matmul(
                        po[:, 0:65],
                        et_tiles[par][t][:, hh, qq * 128 : (qq + 1) * 128],
                        v_sb[t][:, h, 0:65],
                        start=(t == 0),
                        stop=(t == qc),
                    )
                rec = rec_p.tile([128, 1], F32, tag="rec", name="rec")
                nc.vector.reciprocal(rec[:], po[:, 64:65])
                on = on_p.tile(
                    [128, 64], F16, tag=f"dn{m % 2}_{qq}_{hh}", name="dn", bufs=1
                )
                nc.vector.tensor_scalar_mul(on[:], po[:, 0:64], rec[:, 0:1])
                pair.append(on)
            return pair

        def pv_flush(m, j, ons):
            for qp in range(len(ons) // 2):
                trp = ps_tr.tile([128, 2, 128], F16, tag="tr", name="trp")
                for sub in range(2):
                    for hh in range(2):
                        nc.tensor.transpose(
                            trp[64 * hh : 64 * hh + 64, sub, :],
                            ons[2 * qp + sub][hh][:],
                            iden_sb[:],
                            tile_position=(0, 64 * hh),
                        )
                qc = 4 * j + 2 * qp + 1
                nc.vector.tensor_copy(
                    ot_sb[m][:, (qc - 1) * 128 : (qc + 1) * 128],
                    trp[:].rearrange("p a b -> p (a b)"),
                )

        def pv_chunk_defer(m, j):
            ons = [pv_chains(m, j, qq) for qq in range(4)]
            return lambda: pv_flush(m, j, ons)

        def pv_chunk(m, j, qq):
            tg, i0 = m // 2, 2 * (m % 2)
            par = m if j == 0 else m % 2
            h0, h1 = 4 * tg + i0, 4 * tg + i0 + 1
            qc = 4 * j + qq
            # transpose targets pair up (two qc's per [128, 256] fp16 PSUM
            # tile) so the DVE evacuation is one copy per pair; the last
            # pair's O-proj chunks stay per-qc for the tail
            solo = m == 3 and j == ns - 1 and qq >= 2  # tail chunks stream per-qc
            if qq % 2 == 0 or solo:
                trp_cur[m] = ps_tr.tile([128, 2, 128], F16, tag="tr", name="trp")
            trp = trp_cur[m]
            for hh, h in ((0, h0), (1, h1)):
                po = ps_main.tile([128, 512], F32, tag="ps", name="po")
                for t in range(qc + 1):
                    nc.tensor.matmul(
                        po[:, 0:65],
                        et_tiles[par][t][:, hh, qq * 128 : (qq + 1) * 128],
                        v_sb[t][:, h, 0:65],
                        start=(t == 0),
                        stop=(t == qc),
                    )
                rec = rec_p.tile([128, 1], F32, tag="rec", name="rec")
                nc.vector.reciprocal(rec[:], po[:, 64:65])
                on = on_p.tile([128, 64], F16, tag="on", name="on")
                nc.vector.tensor_scalar_mul(on[:], po[:, 0:64], rec[:, 0:1])
                nc.tensor.transpose(
                    trp[64 * hh : 64 * hh + 64, 0 if solo else qq % 2, :],
                    on[:],
                    iden_sb[:],
                    tile_position=(0, 64 * hh),
                )
            if solo:
                nc.vector.tensor_copy(
                    ot_sb[m][:, qc * 128 : (qc + 1) * 128], trp[:, 0, :]
                )
                oproj_chunk(j, qq, act_evac=True)
            elif qq % 2 == 1:
                nc.vector.tensor_copy(
                    ot_sb[m][:, (qc - 1) * 128 : (qc + 1) * 128],
                    trp[:].rearrange("p a b -> p (a b)"),
                )
                if m == 3 and j == ns - 1:
                    oproj_chunk(j, qq - 1)
                    oproj_chunk(j, qq)

        def pv_block(m, j):
            pv_chunk_defer(m, j)()

        def scores_pv_block(m, j, mid=None, post=None):
            """Scores with the PV chunks woven between the diagonal t's so
            the window tail overlaps PE work with the last exps.  `post`
            (next window's first scores) lands after the last woven one."""
            for t in range(4 * j + 1):
                scores_t(m, j, t)
            if mid is not None:
                mid()
            if j == ns - 1:
                # tail: inline transposes/oproj so output chunks stream
                for qq in range(4):
                    if qq < 3:
                        scores_t(m, j, 4 * j + 1 + qq)
                    pv_chunk(m, j, qq)
                    if qq == 2 and post is not None:
                        post()
            else:
                ons = []
                for qq in range(4):
                    if qq < 3:
                        scores_t(m, j, 4 * j + 1 + qq)
                    ons.append(pv_chains(m, j, qq))
                    if qq == 2 and post is not None:
                        post()
                pv_flush(m, j, ons)

        def oproj_chunk(j, qq, act_evac=False):
            st = 4 * j + qq
            for dsl in range(2):
                po2 = ps_main.tile([128, 512], F32, tag="ps", name="po2")
                for cc in range(4):
                    nc.tensor.matmul(
                        po2[:],
                        ot_sb[cc][:, st * 128 : (st + 1) * 128],
                        wo_sb[:, cc, dsl * 512 : (dsl + 1) * 512],
                        start=(cc == 0),
                        stop=(cc == 3),
                    )
                osb = osb_p.tile([128, 512], F16, tag="osb", name="osb")
                if act_evac and dsl == 1:
                    # post-exp tail: ScalarE is idle, split the serial
                    # evacuation chain across both engines
                    nc.scalar.activation(osb[:], po2[:], AF.Copy)
                else:
                    nc.vector.tensor_copy(osb[:], po2[:])
                nc.sync.dma_start(
                    out=out_d[
                        st * 128 : (st + 1) * 128, dsl * 512 : (dsl + 1) * 512
                    ],
                    in_=osb[:],
                )

        # --- emission: prologue + j-outer / pair-inner windows ---
        # Projection-group units: (tg, sl) -> 4 groups (Q i0/i1, K i0/i1).
        # Emitted lazily as PE fillers; ensure() forces a unit's deadline.
        unit_q = [(0, 0)] + [
            (tg, sl) for sl in range(ns) for tg in range(2) if (tg, sl) != (0, 0)
        ]
        done_units = set()

        def emit_unit(tg, sl, act_evac=False):
            for pi in range(2):
                for inst in range(2):
                    qk_group(pi, tg, inst, sl, act_evac=act_evac)
            done_units.add((tg, sl))

        def ensure(tg, sl):
            while (tg, sl) not in done_units:
                fill_unit()

        def fill_unit():
            for u in unit_q:
                if u not in done_units:
                    emit_unit(*u)
                    return

        emit_unit(0, 0, act_evac=True)  # prologue: K evacs on idle ScalarE

        def oproj_block(j):
            for qq in range(4):
                oproj_chunk(j, qq)

        # Window 0: V (fp16 xt) lands late in the DMA stream and the window
        # has little exp work, so all four pairs' scores are front-loaded,
        # V chunks woven into pair 3's diagonal, and window 1's first two
        # score blocks pulled in to cover the PV tail.  O-projections are
        # deferred two windows (PE work moves into the ACT-bound windows).
        scores_block(0, 0)
        scores_block(1, 0)
        ensure(1, 0)
        if ns > 1:
            ensure(0, 1)
        fill_unit()  # (1,1): unblocks window 1 while PE/DVE are idle
        scores_block(2, 0)
        scores_t(3, 0, 0)
        scores_t(3, 0, 1)
        scores_t(3, 0, 2)
        scores_t(3, 0, 3)
        v_group_multi([0, 1, 2])
        v_group(3)
        if ns > 1:
            flush0 = pv_chunk_defer(0, 0)
            scores_block(0, 1)
            flush0()
        else:
            pv_block(0, 0)
        pv_block(1, 0)
        if ns > 1:
            scores_block(1, 1)
            pv_block(2, 0)
            v_group(4)
            v_group(5)
        else:
            pv_block(2, 0)
            for qq in range(4):
                pv_chunk(3, 0, qq)

        for j in range(1, ns):
            if j > 1:
                v_group(4 * j)  # scores_block(0, j) was pulled into the
                scores_block(1, j)  # previous window's weave
                v_group(4 * j + 1)
            ensure(1, j)
            v_group(4 * j + 2)
            v_group(4 * j + 3)
            pv_block(0, j)
            if j >= 2:
                oproj_block(j - 2)
            scores_block(2, j)
            if j == ns - 1 and j >= 1:
                oproj_block(j - 1)
            pv_block(1, j)
            if j == 1:
                pv_block(3, 0)
            last = j == ns - 1

            def mid(j=j):
                pv_block(2, j)

            def post(j=j, last=last):
                if not last:
                    ensure(0, j + 1)
                    scores_block(0, j + 1)

            scores_pv_block(3, j, mid=mid, post=post)

    if legalize:
        _legalize_waits(nc)
    return nc


_NC_CACHE = {}
FP8_PROJ = True


def _get_nc(s=S):
    key = (s, FP8_PROJ)
    if key not in _NC_CACHE:
        _NC_CACHE[key] = build_nc(s, fp8_proj=FP8_PROJ)
    return _NC_CACHE[key]


def _perm(lo):
    """Column permutation for Wq/Wk: new col a = tg*256 + inst*128 + p maps
    to original output dim lo + 64*(4*tg + p//32) + inst*32 + p%32."""
    a = np.arange(GW)
    tg, r = a // 256, a % 256
    inst, p = r // 128, r % 128
    return lo + 64 * (4 * tg + p // 32) + inst * 32 + (p % 32)


def _to_f8(a):
    import ml_dtypes

    return a.astype(ml_dtypes.float8_e4m3)


def make_inputs(X, Wq, bq, Wk, bk, Wv, bv, Wo, bo, s=S, fp8_proj=False):
    """Per-core input maps. Core c: batch c//2, head group c%2."""
    iv, jv = np.arange(128)[:, None], np.arange(128)[None, :]
    mask = np.where(jv >= iv, 0.0, -30000.0).astype(np.float16)
    iden = np.eye(128, dtype=np.float16)
    nd = D // 128
    in_maps = []
    for c in range(8):
        b, g = divmod(c, 2)
        lo, hi = g * GW, (g + 1) * GW
        perm = _perm(lo)
        bqk = np.empty((128, 8), np.float32)
        for pi, bias in enumerate((bq, bk)):
            for tg in range(2):
                for inst in range(2):
                    cols = perm[tg * 256 + inst * 128 : tg * 256 + inst * 128 + 128]
                    bqk[:, pi * 4 + tg * 2 + inst] = bias[cols]
        xt = np.ascontiguousarray(X[b, :s].T).astype(np.float16)
        m = {
            "xt": xt,
            "wv": np.ascontiguousarray(Wv[lo:hi].T).astype(np.float16),
            "wo": np.ascontiguousarray(Wo[:, lo:hi].T).astype(np.float16),
            "bqk": bqk,
            "bvb": np.tile(bv[lo:hi].astype(np.float16), (128, 1)),
            "mask": mask,
            "iden": iden,
        }
        if fp8_proj:
            # xt8 [128, nd, s]: [p, d, s] = X.T[d*128+p, s]
            m["xt8"] = _to_f8(xt.reshape(nd, 128, s).transpose(1, 0, 2))
            for nm, W in (("wq", Wq), ("wk", Wk)):
                wt = np.ascontiguousarray(W[perm].T)  # [D, GW]
                # [128, nd//2, 2, GW]: [p, dd, i, c] = wt[(2dd+i)*128+p, c]
                m[nm] = _to_f8(
                    wt.reshape(nd // 2, 2, 128, GW).transpose(2, 0, 1, 3)
                )
        else:
            m["wq"] = np.ascontiguousarray(Wq[perm].T).astype(np.float16)
            m["wk"] = np.ascontiguousarray(Wk[perm].T).astype(np.float16)
        in_maps.append(m)
    return in_maps


def kernel(X, Wq, bq, Wk, bk, Wv, bv, Wo, bo, **run_kwargs):
    args = [np.asarray(a, np.float32) for a in (X, Wq, bq, Wk, bk, Wv, bv, Wo, bo)]
    X, Wq, bq, Wk, bk, Wv, bv, Wo, bo = args
    nc = _get_nc(S)
    in_maps = make_inputs(X, Wq, bq, Wk, bk, Wv, bv, Wo, bo, S, fp8_proj=FP8_PROJ)
    res = run_bass_kernel_spmd(nc, in_maps, core_ids=list(range(8)), **run_kwargs)
    outs = [r["out"] for r in res.results]
    full = np.empty((B, S, D), np.float32)
    for b in range(B):
        full[b] = outs[2 * b].astype(np.float32) + outs[2 * b + 1].astype(np.float32) + bo
    kernel.last_results = res
    return full


# revision 68
# speedup vs baseline: 1.0037x; 1.0002x over previous
"""Multi-head causal attention (B=4, S=2048, D=1024, H=16) on 8 NeuronCores.

Sharding: core c handles batch b = c//2 and head-group g = c%2 (8 heads).
Each core computes QKV projections for its group, causal attention for its
8 heads, and a partial output projection (row-split Wo).  Host sums the two
fp16 partials per batch in fp32 and adds bo.

On-chip design (per core):
  XT [D, S] = X[b].T fp16 in 8 chunks [128, S].
  Q/K are projected (stationary W-chunk x moving XT) and evacuated from
  PSUM straight to fp8e4 tiles qt8/kt8 [128, 2, S] (Q with a fused
  bias-add; K takes none - softmax is invariant to the key bias, so K
  evacs are plain casts and the prologue's run on the then-idle ScalarE):
  partition p of tile-group tg holds head 4*tg + p//32, dim1 selects the
  hd half (hd = dim1*32 + p%32).  Host permutes Wq/Wk columns to produce
  this layout directly.
  Scores run as fp8 DoubleRow matmuls (2 k-tiles of 32 partitions), giving
  qk [128 k, 2 heads, 512 q] PSUM blocks at half the fp16 cycle cost; exp
  on ScalarE (scale=1/8, no max subtraction; |scores| <= ~3) into fp16 et
  tiles (tag sets alternate by pair parity so the next pair's exps overlap
  the previous pair's PV reads; window 0 gets one set per pair).  Causal
  masking of diagonal 128x128 blocks happens inside the scores PSUM
  accumulation: an identity-stationary matmul adds 0/-30000 so exp zeroes
  the upper triangle with no extra vector pass.
  PV is TRANSPOSED: po [q=128, 65] += et_t[:, hh, qcol].T @ v_t[:, h, 0:65]
  accumulated over t (col 64 of v is ones and yields the exp rowsum), so
  the softmax divisor lands in a PSUM column: DVE reciprocal of po[:,64]
  then a per-partition tensor_scalar multiply evacuates the normalized
  [q, 64] tile in one shot (no scatter/gather/broadcast DMAs).  PV blocks
  emit all chains+norms first and the PE transposes (identity matmul,
  fp16 PSUM, two heads via tile_position col 0/64, two q-chunks per tile)
  afterwards, keeping DVE round-trip waits off the PE queue.
  O-projection runs at st-chunk granularity, evacuated fp32->fp16 on DVE
  (ScalarE Copy for the tail chunks), stored via per-half [128, 512] fp16
  DMAs; each window's O-projection is deferred two windows so the PE work
  lands in ACT-bound stretches.
  Emission is j-window-outer / pair-inner with V-projection chunks and
  Q/K projection groups interleaved as PE fillers, window 1's first score
  blocks pulled into window 0's tail, and each next window's first score
  block pulled into the previous weave, so ScalarE stays fed.

Walrus wait-slot legality (1 sem wait per engine instruction): Tile's
extra waits are split onto same-engine NoOps by _legalize_waits.
"""

import sys

for _p in ("/opt/trn_rl_repo",):
    if _p not in sys.path:
        sys.path.insert(0, _p)

from contextlib import ExitStack

import numpy as np

import concourse.bass as bass
import concourse.mybir as mybir
import concourse.tile as tile
from concourse.bass_utils import run_bass_kernel_spmd

import bass_rust

F16 = mybir.dt.float16
F32 = mybir.dt.float32
F8 = mybir.dt.float8e4
AF = mybir.ActivationFunctionType
DR = mybir.MatmulPerfMode.DoubleRow

B, S, D, H = 4, 2048, 1024, 16
HD = D // H  # 64
GH = 8  # heads per group
GW = GH * HD  # 512 columns per group


_SPLITTABLE = {
    "InstMatmult", "InstLdweights", "InstActivation", "InstTensorCopy",
    "InstTensorTensor", "InstTensorScalarPtr", "InstTensorReduce",
    "InstMemset", "InstDMACopy", "InstReciprocal", "InstIota",
    "InstTensorTensorReduce", "InstBNStats", "InstBNStatsAggregate",
    "InstStreamShuffle", "InstNoOp", "InstPool", "InstMax", "InstDrain",
}


def _legalize_waits(nc, max_waits=1):
    """Walrus codegen accepts at most one sync-wait command per engine
    instruction; Tile's wait assigner can emit more.  Split extras onto
    same-engine NoOps inserted immediately before (semantics preserved:
    the engine blocks at the same program point)."""
    ctr = 0
    for fn in nc.m.functions:
        for blk in fn.blocks:
            out = []
            for ins in blk.instructions:
                si = ins.sync_info
                if (
                    si is not None
                    and len(si.on_wait) > max_waits
                    and type(ins).__name__ in _SPLITTABLE
                ):
                    waits = list(si.on_wait)
                    extra, keep = waits[:-max_waits], waits[-max_waits:]
                    for w in extra:
                        nop = mybir.InstNoOp(name=f"waitnop-{ctr}", ins=[], outs=[])
                        ctr += 1
                        nop.engine = ins.engine
                        nop.sync_info = bass_rust.SyncInfo(on_wait=[w], on_update=[])
                        out.append(nop)
                    ins.sync_info = bass_rust.SyncInfo(
                        on_wait=keep, on_update=list(si.on_update)
                    )
                out.append(ins)
            blk.instructions[:] = out
    return ctr


def build_nc(s=S, fp8_proj=False, legalize=True):
    ns = s // 512  # 512-wide q windows
    nt = s // 128  # 128-wide s chunks
    nd = D // 128  # contraction chunks for projections

    nc = bass.Bass("TRN2", target_bir_lowering=False, debug=False)
    xt_d = nc.dram_tensor("xt", [D, s], F16, kind="ExternalInput").ap()
    if fp8_proj:
        xt8_d = nc.dram_tensor("xt8", [128, nd, s], F8, kind="ExternalInput").ap()
        wq_d = nc.dram_tensor("wq", [128, nd // 2, 2, GW], F8, kind="ExternalInput").ap()
        wk_d = nc.dram_tensor("wk", [128, nd // 2, 2, GW], F8, kind="ExternalInput").ap()
    else:
        wq_d = nc.dram_tensor("wq", [D, GW], F16, kind="ExternalInput").ap()
        wk_d = nc.dram_tensor("wk", [D, GW], F16, kind="ExternalInput").ap()
    wv_d = nc.dram_tensor("wv", [D, GW], F16, kind="ExternalInput").ap()
    wo_d = nc.dram_tensor("wo", [GW, D], F16, kind="ExternalInput").ap()
    bqk_d = nc.dram_tensor("bqk", [128, 8], F32, kind="ExternalInput").ap()
    bvb_d = nc.dram_tensor("bvb", [128, GW], F16, kind="ExternalInput").ap()
    mask_d = nc.dram_tensor("mask", [128, 128], F16, kind="ExternalInput").ap()
    iden_d = nc.dram_tensor("iden", [128, 128], F16, kind="ExternalInput").ap()
    out_d = nc.dram_tensor("out", [s, D], F16, kind="ExternalOutput").ap()

    with tile.TileContext(nc) as tc, ExitStack() as ctx:
        pool = lambda name, bufs, **kw: ctx.enter_context(
            tc.tile_pool(name=name, bufs=bufs, **kw)
        )
        const_p = pool("const", 1)
        xt_p = pool("xtp", nd)
        w_p = pool("wp", 1)
        qkt_p = pool("qktp", 1)
        v_p = pool("vp", nt)
        et_p = pool("etp", 1)  # tags e{parity}_{t}, one buf each
        ot_p = pool("otp", 4)
        on_p = pool("onp", 3)
        rec_p = pool("recp", 3)
        osb_p = pool("osbp", 3)
        ps_qk = pool("psqk", 2, space="PSUM")      # [128,2,512] f32 -> 4 banks
        ps_main = pool("psmain", 3, space="PSUM")  # [128,512] f32 -> 3 banks
        ps_tr = pool("pstr", 1, space="PSUM")      # [128,128] f16 -> 1 bank

        # --- input DMAs, in order of first use ---
        bqk_sb = const_p.tile([128, 8], F32)
        nc.sync.dma_start(out=bqk_sb[:], in_=bqk_d[:])
        if fp8_proj:
            xt8_sb = w_p.tile([128, nd, s], F8, name="xt8")
            nc.sync.dma_start(out=xt8_sb[:, 0:2, :], in_=xt8_d[:, 0:2, :])
            wq_sb = w_p.tile([128, nd // 2, 2, GW], F8, name="wq8")
            wk_sb = w_p.tile([128, nd // 2, 2, GW], F8, name="wk8")
            nc.sync.dma_start(out=wq_sb[:], in_=wq_d[:])
            nc.sync.dma_start(out=wk_sb[:], in_=wk_d[:])
            for dd in range(1, nd // 2):
                nc.sync.dma_start(
                    out=xt8_sb[:, 2 * dd : 2 * dd + 2, :],
                    in_=xt8_d[:, 2 * dd : 2 * dd + 2, :],
                )
            mask_sb = const_p.tile([128, 128], F16)
            nc.sync.dma_start(out=mask_sb[:], in_=mask_d[:])
            iden_sb = const_p.tile([128, 128], F16)
            nc.sync.dma_start(out=iden_sb[:], in_=iden_d[:])
            bvb_sb = const_p.tile([128, GW], F16)
            nc.sync.dma_start(out=bvb_sb[:], in_=bvb_d[:])
            wv_sb = w_p.tile([128, nd, GW], F16, name="wv")
            nc.sync.dma_start(out=wv_sb[:], in_=wv_d.rearrange("(d p) n -> p d n", p=128))
            xt_sb = []
            for d in range(nd):
                t = xt_p.tile([128, s], F16, tag="xt", name=f"xtc{d}")
                nc.sync.dma_start(out=t[:], in_=xt_d[d * 128 : (d + 1) * 128, :])
                xt_sb.append(t)
        else:
            wq_sb = w_p.tile([128, nd, GW], F16, name="wq")
            nc.sync.dma_start(out=wq_sb[:], in_=wq_d.rearrange("(d p) n -> p d n", p=128))
            xt_sb = []
            for d in range(nd):
                t = xt_p.tile([128, s], F16, tag="xt", name=f"xtc{d}")
                nc.sync.dma_start(out=t[:], in_=xt_d[d * 128 : (d + 1) * 128, :])
                xt_sb.append(t)
            wk_sb = w_p.tile([128, nd, GW], F16, name="wk")
            nc.sync.dma_start(out=wk_sb[:], in_=wk_d.rearrange("(d p) n -> p d n", p=128))
            bvb_sb = const_p.tile([128, GW], F16)
            nc.sync.dma_start(out=bvb_sb[:], in_=bvb_d[:])
            mask_sb = const_p.tile([128, 128], F16)
            nc.sync.dma_start(out=mask_sb[:], in_=mask_d[:])
            wv_sb = w_p.tile([128, nd, GW], F16, name="wv")
            nc.sync.dma_start(out=wv_sb[:], in_=wv_d.rearrange("(d p) n -> p d n", p=128))
        if not fp8_proj:
            iden_sb = const_p.tile([128, 128], F16)
            nc.sync.dma_start(out=iden_sb[:], in_=iden_d[:])
        wo_sb = w_p.tile([128, 4, D], F16, name="wo")
        nc.sync.dma_start(out=wo_sb[:], in_=wo_d.rearrange("(c p) n -> p c n", p=128))

        # touch ops: early Exp ACT-table load + const observations
        scr_a = const_p.tile([128, 1], F32)
        nc.scalar.activation(scr_a[:], bqk_sb[:, 0:1], AF.Exp, scale=1.0)
        scr_v = const_p.tile([128, 1], F16)
        nc.vector.tensor_copy(scr_v[:], bvb_sb[:, 0:1])
        scr_m = const_p.tile([128, 1], F16)
        nc.vector.tensor_copy(scr_m[:], mask_sb[:, 0:1])
        scr_i = const_p.tile([128, 1], F16)
        nc.gpsimd.tensor_copy(scr_i[:], iden_sb[:, 0:1])

        ot_sb = [ot_p.tile([128, s], F16, tag="ot", name=f"ot{m}") for m in range(4)]
        qt8 = [qkt_p.tile([128, 2, s], F8, name=f"qt8_{tg}") for tg in range(2)]
        kt8 = [qkt_p.tile([128, 2, s], F8, name=f"kt8_{tg}") for tg in range(2)]

        # --- projection-group emitters (used as PE fillers) ---
        def qk_group(pi, tg, inst, sl, act_evac=False):
            """One Q/K projection PSUM group -> fp8 evacuation.  Q gets the
            bias folded in; K needs none (softmax is invariant to the key
            bias: the q.bk term is constant per query row and cancels in
            normalization), which also lets prologue K evacs run on the
            then-idle ScalarE."""
            dst = (qt8, kt8)[pi][tg]
            col = (tg * 2 + inst) * 128
            ps = ps_main.tile([128, 512], F32, tag="ps", name="ps")
            if fp8_proj:
                for dd in range(nd // 2):
                    nc.tensor.matmul(
                        ps[:],
                        wq_sb[:, dd, :, col : col + 128] if pi == 0
                        else wk_sb[:, dd, :, col : col + 128],
                        xt8_sb[:, 2 * dd : 2 * dd + 2, sl * 512 : (sl + 1) * 512],
                        start=(dd == 0),
                        stop=(dd == nd // 2 - 1),
                        perf_mode=DR,
                    )
            else:
                for d in range(nd):
                    nc.tensor.matmul(
                        ps[:],
                        (wq_sb, wk_sb)[pi][:, d, col : col + 128],
                        xt_sb[d][:, sl * 512 : (sl + 1) * 512],
                        start=(d == 0),
                        stop=(d == nd - 1),
                    )
            dslice = dst[:, inst, sl * 512 : (sl + 1) * 512]
            if pi == 1:
                if act_evac:
                    nc.scalar.activation(dslice, ps[:], AF.Copy)
                else:
                    nc.vector.tensor_copy(dslice, ps[:])
            else:
                bc = tg * 2 + inst
                nc.vector.tensor_scalar_add(dslice, ps[:], bqk_sb[:, bc : bc + 1])

        v_sb = [None] * nt

        def v_group_multi(sts):
            """d-major interleave across <=3 chunks so only the last
            d-matmul of each chain waits the final xt DMA."""
            pss = []
            for st in sts:
                pss.append(ps_main.tile([128, 512], F32, tag="ps", name="ps"))
            for d in range(nd):
                for ps, st in zip(pss, sts):
                    nc.tensor.matmul(
                        ps[:],
                        xt_sb[d][:, st * 128 : (st + 1) * 128],
                        wv_sb[:, d, :],
                        start=(d == 0),
                        stop=(d == nd - 1),
                    )
            for ps, st in zip(pss, sts):
                vt = v_p.tile([128, GH, 65], F16, tag="v", name=f"v{st}")
                nc.vector.memset(vt[:, :, 64:65], 1.0)
                nc.vector.tensor_add(
                    vt[:, :, 0:64],
                    ps[:].rearrange("p (h e) -> p h e", h=GH),
                    bvb_sb[:].rearrange("p (h e) -> p h e", h=GH),
                )
                v_sb[st] = vt

        def v_group(st):
            """One V-projection s-chunk [128, 8, 65] with ones col."""
            ps = ps_main.tile([128, 512], F32, tag="ps", name="ps")
            for d in range(nd):
                nc.tensor.matmul(
                    ps[:],
                    xt_sb[d][:, st * 128 : (st + 1) * 128],
                    wv_sb[:, d, :],
                    start=(d == 0),
                    stop=(d == nd - 1),
                )
            vt = v_p.tile([128, GH, 65], F16, tag="v", name=f"v{st}")
            nc.vector.memset(vt[:, :, 64:65], 1.0)
            nc.vector.tensor_add(
                vt[:, :, 0:64],
                ps[:].rearrange("p (h e) -> p h e", h=GH),
                bvb_sb[:].rearrange("p (h e) -> p h e", h=GH),
            )
            v_sb[st] = vt

        # --- attention building blocks ---
        et_tiles = [[None] * nt for _ in range(4)]  # by pair parity (j0: per pair)

        def scores_t(m, j, t):
            tg, i0 = m // 2, 2 * (m % 2)
            par = m if j == 0 else m % 2
            diag = t >= 4 * j
            w0 = 128 * (t - 4 * j) if diag else 0
            qk = ps_qk.tile([128, 2, 512], F32, tag="qk", name="qk")
            for hh in range(2):
                ib = 32 * (i0 + hh)
                nc.tensor.matmul(
                    qk[:, hh, w0:512],
                    kt8[tg][ib : ib + 32, :, t * 128 : (t + 1) * 128],
                    qt8[tg][ib : ib + 32, :, j * 512 + w0 : (j + 1) * 512],
                    start=True,
                    stop=not diag,
                    perf_mode=DR,
                    tile_position=(ib, 0),
                    skip_group_check=diag,
                )
                if diag:
                    # causal mask: accumulate 0/-30000 into the diagonal
                    # 128x128 block (identity-stationary matmul) so exp
                    # zeroes the upper triangle with no DVE pass
                    nc.tensor.matmul(
                        qk[:, hh, w0 : w0 + 128],
                        iden_sb[:],
                        mask_sb[:],
                        start=False,
                        stop=True,
                        skip_group_check=True,
                    )
            et = et_p.tile([128, 2, 512], F16, tag=f"e{par}_{t}", name=f"e{par}_{t}")
            nc.scalar.activation(
                et[:, :, w0:512], qk[:, :, w0:512], AF.Exp, scale=0.125
            )
            et_tiles[par][t] = et

        def scores_block(m, j):
            for t in range(4 * j + 4):
                scores_t(m, j, t)

        trp_cur = {}

        def pv_chains(m, j, qq):
            """PV chains + norms only (no PE-blocking transpose waits)."""
            tg, i0 = m // 2, 2 * (m % 2)
            par = m if j == 0 else m % 2
            h0, h1 = 4 * tg + i0, 4 * tg + i0 + 1
            qc = 4 * j + qq
            pair = []
            for hh, h in ((0, h0), (1, h1)):
                po = ps_main.tile([128, 512], F32, tag="ps", name="po")
                for t in range(qc + 1):
                    nc.tensor.matmul(
                        po[:, 0:65],
                        et_tiles[par][t][:, hh, qq * 128 : (qq + 1) * 128],
                        v_sb[t][:, h, 0:65],
                        start=(t == 0),
                        stop=(t == qc),
                    )
                rec = rec_p.tile([128, 1], F32, tag="rec", name="rec")
                nc.vector.reciprocal(rec[:], po[:, 64:65])
                on = on_p.tile(
                    [128, 64], F16, tag=f"dn{m % 2}_{qq}_{hh}", name="dn", bufs=1
                )
                nc.vector.tensor_scalar_mul(on[:], po[:, 0:64], rec[:, 0:1])
                pair.append(on)
            return pair

        def pv_flush(m, j, ons, qq0=0):
            for qp in range(len(ons) // 2):
                trp = ps_tr.tile([128, 2, 128], F16, tag="tr", name="trp")
                for sub in range(2):
                    for hh in range(2):
                        nc.tensor.transpose(
                            trp[64 * hh : 64 * hh + 64, sub, :],
                            ons[2 * qp + sub][hh][:],
                            iden_sb[:],
                            tile_position=(0, 64 * hh),
                        )
                qc = 4 * j + qq0 + 2 * qp + 1
                nc.vector.tensor_copy(
                    ot_sb[m][:, (qc - 1) * 128 : (qc + 1) * 128],
                    trp[:].rearrange("p a b -> p (a b)"),
                )

        def pv_chunk_defer(m, j):
            ons = [pv_chains(m, j, qq) for qq in range(4)]
            return lambda: pv_flush(m, j, ons)

        def pv_chunk(m, j, qq):
            tg, i0 = m // 2, 2 * (m % 2)
            par = m if j == 0 else m % 2
            h0, h1 = 4 * tg + i0, 4 * tg + i0 + 1
            qc = 4 * j + qq
            # transpose targets pair up (two qc's per [128, 256] fp16 PSUM
            # tile) so the DVE evacuation is one copy per pair; the last
            # pair's O-proj chunks stay per-qc for the tail
            solo = m == 3 and j == ns - 1 and qq >= 2  # tail chunks stream per-qc
            if qq % 2 == 0 or solo:
                trp_cur[m] = ps_tr.tile([128, 2, 128], F16, tag="tr", name="trp")
            trp = trp_cur[m]
            for hh, h in ((0, h0), (1, h1)):
                po = ps_main.tile([128, 512], F32, tag="ps", name="po")
                for t in range(qc + 1):
                    nc.tensor.matmul(
                        po[:, 0:65],
                        et_tiles[par][t][:, hh, qq * 128 : (qq + 1) * 128],
                        v_sb[t][:, h, 0:65],
                        start=(t == 0),
                        stop=(t == qc),
                    )
                rec = rec_p.tile([128, 1], F32, tag="rec", name="rec")
                nc.vector.reciprocal(rec[:], po[:, 64:65])
                on = on_p.tile([128, 64], F16, tag="on", name="on")
                nc.vector.tensor_scalar_mul(on[:], po[:, 0:64], rec[:, 0:1])
                nc.tensor.transpose(
                    trp[64 * hh : 64 * hh + 64, 0 if solo else qq % 2, :],
                    on[:],
                    iden_sb[:],
                    tile_position=(0, 64 * hh),
                )
            if solo:
                nc.vector.tensor_copy(
                    ot_sb[m][:, qc * 128 : (qc + 1) * 128], trp[:, 0, :]
                )
                oproj_chunk(j, qq, act_evac=True)
            elif qq % 2 == 1:
                nc.vector.tensor_copy(
                    ot_sb[m][:, (qc - 1) * 128 : (qc + 1) * 128],
                    trp[:].rearrange("p a b -> p (a b)"),
                )
                if m == 3 and j == ns - 1:
                    oproj_chunk(j, qq - 1)
                    oproj_chunk(j, qq)

        def pv_block(m, j):
            pv_chunk_defer(m, j)()

        def scores_pv_block(m, j, mid=None, post=None):
            """Scores with the PV chunks woven between the diagonal t's so
            the window tail overlaps PE work with the last exps.  `post`
            (next window's first scores) lands after the last woven one."""
            for t in range(4 * j + 1):
                scores_t(m, j, t)
            if mid is not None:
                mid()
            if j == ns - 1:
                # tail: chains inline, pair-flush + oproj after the next
                # woven score so no DVE transpose-wait precedes an exp
                ons = []
                for qq in range(4):
                    if qq < 3:
                        scores_t(m, j, 4 * j + 1 + qq)
                    ons.append(pv_chains(m, j, qq))
                    if qq == 2 and post is not None:
                        post()
                    if qq % 2 == 1:
                        pv_flush(m, j, ons[qq - 1 : qq + 1], qq0=qq - 1)
                        oproj_chunk(j, qq - 1, act_evac=qq == 3)
                        oproj_chunk(j, qq, act_evac=qq == 3)
            else:
                ons = []
                for qq in range(4):
                    if qq < 3:
                        scores_t(m, j, 4 * j + 1 + qq)
                    ons.append(pv_chains(m, j, qq))
                    if qq == 2 and post is not None:
                        post()
                pv_flush(m, j, ons)

        def oproj_chunk(j, qq, act_evac=False):
            st = 4 * j + qq
            for dsl in range(2):
                po2 = ps_main.tile([128, 512], F32, tag="ps", name="po2")
                for cc in range(4):
                    nc.tensor.matmul(
                        po2[:],
                        ot_sb[cc][:, st * 128 : (st + 1) * 128],
                        wo_sb[:, cc, dsl * 512 : (dsl + 1) * 512],
                        start=(cc == 0),
                        stop=(cc == 3),
                    )
                osb = osb_p.tile([128, 512], F16, tag="osb", name="osb")
                if act_evac and dsl == 1:
                    # post-exp tail: ScalarE is idle, split the serial
                    # evacuation chain across both engines
                    nc.scalar.activation(osb[:], po2[:], AF.Copy)
                else:
                    nc.vector.tensor_copy(osb[:], po2[:])
                nc.sync.dma_start(
                    out=out_d[
                        st * 128 : (st + 1) * 128, dsl * 512 : (dsl + 1) * 512
                    ],
                    in_=osb[:],
                )

        # --- emission: prologue + j-outer / pair-inner windows ---
        # Projection-group units: (tg, sl) -> 4 groups (Q i0/i1, K i0/i1).
        # Emitted lazily as PE fillers; ensure() forces a unit's deadline.
        unit_q = [(0, 0)] + [
            (tg, sl) for sl in range(ns) for tg in range(2) if (tg, sl) != (0, 0)
        ]
        done_units = set()

        def emit_unit(tg, sl, act_evac=False):
            for pi in range(2):
                for inst in range(2):
                    qk_group(pi, tg, inst, sl, act_evac=act_evac)
            done_units.add((tg, sl))

        def ensure(tg, sl):
            while (tg, sl) not in done_units:
                fill_unit()

        def fill_unit():
            for u in unit_q:
                if u not in done_units:
                    emit_unit(*u)
                    return

        emit_unit(0, 0, act_evac=True)  # prologue: K evacs on idle ScalarE

        def oproj_block(j):
            for qq in range(4):
                oproj_chunk(j, qq)

        # Window 0: V (fp16 xt) lands late in the DMA stream and the window
        # has little exp work, so all four pairs' scores are front-loaded,
        # V chunks woven into pair 3's diagonal, and window 1's first two
        # score blocks pulled in to cover the PV tail.  O-projections are
        # deferred two windows (PE work moves into the ACT-bound windows).
        scores_block(0, 0)
        scores_block(1, 0)
        ensure(1, 0)
        if ns > 1:
            ensure(0, 1)
        fill_unit()  # (1,1): unblocks window 1 while PE/DVE are idle
        scores_block(2, 0)
        scores_t(3, 0, 0)
        scores_t(3, 0, 1)
        scores_t(3, 0, 2)
        scores_t(3, 0, 3)
        v_group_multi([0, 1, 2])
        v_group(3)
        if ns > 1:
            flush0 = pv_chunk_defer(0, 0)
            scores_block(0, 1)
            flush0()
        else:
            pv_block(0, 0)
        pv_block(1, 0)
        if ns > 1:
            scores_block(1, 1)
            pv_block(2, 0)
            v_group(4)
            v_group(5)
        else:
            pv_block(2, 0)
            for qq in range(4):
                pv_chunk(3, 0, qq)

        for j in range(1, ns):
            if j > 1:
                scores_block(1, j)  # scores_block(0, j) was pulled into
                v_group(4 * j)      # the previous window's weave
                v_group(4 * j + 1)
            ensure(1, j)
            v_group(4 * j + 2)
            v_group(4 * j + 3)
            pv_block(0, j)
            if j >= 2:
                oproj_block(j - 2)
            scores_block(2, j)
            if j == ns - 1 and j >= 1:
                oproj_block(j - 1)
            pv_block(1, j)
            if j == 1:
                pv_block(3, 0)
            last = j == ns - 1

            def mid(j=j):
                pv_block(2, j)

            def post(j=j, last=last):
                if not last:
                    ensure(0, j + 1)
                    scores_block(0, j + 1)

            scores_pv_block(3, j, mid=mid, post=post)

    if legalize:
        _legalize_waits(nc)
    return nc


_NC_CACHE = {}
FP8_PROJ = True


def _get_nc(s=S):
    key = (s, FP8_PROJ)
    if key not in _NC_CACHE:
        _NC_CACHE[key] = build_nc(s, fp8_proj=FP8_PROJ)
    return _NC_CACHE[key]


def _perm(lo):
    """Column permutation for Wq/Wk: new col a = tg*256 + inst*128 + p maps
    to original output dim lo + 64*(4*tg + p//32) + inst*32 + p%32."""
    a = np.arange(GW)
    tg, r = a // 256, a % 256
    inst, p = r // 128, r % 128
    return lo + 64 * (4 * tg + p // 32) + inst * 32 + (p % 32)


def _to_f8(a):
    import ml_dtypes

    return a.astype(ml_dtypes.float8_e4m3)


def make_inputs(X, Wq, bq, Wk, bk, Wv, bv, Wo, bo, s=S, fp8_proj=False):
    """Per-core input maps. Core c: batch c//2, head group c%2."""
    iv, jv = np.arange(128)[:, None], np.arange(128)[None, :]
    mask = np.where(jv >= iv, 0.0, -30000.0).astype(np.float16)
    iden = np.eye(128, dtype=np.float16)
    nd = D // 128
    in_maps = []
    for c in range(8):
        b, g = divmod(c, 2)
        lo, hi = g * GW, (g + 1) * GW
        perm = _perm(lo)
        bqk = np.empty((128, 8), np.float32)
        for pi, bias in enumerate((bq, bk)):
            for tg in range(2):
                for inst in range(2):
                    cols = perm[tg * 256 + inst * 128 : tg * 256 + inst * 128 + 128]
                    bqk[:, pi * 4 + tg * 2 + inst] = bias[cols]
        xt = np.ascontiguousarray(X[b, :s].T).astype(np.float16)
        m = {
            "xt": xt,
            "wv": np.ascontiguousarray(Wv[lo:hi].T).astype(np.float16),
            "wo": np.ascontiguousarray(Wo[:, lo:hi].T).astype(np.float16),
            "bqk": bqk,
            "bvb": np.tile(bv[lo:hi].astype(np.float16), (128, 1)),
            "mask": mask,
            "iden": iden,
        }
        if fp8_proj:
            # xt8 [128, nd, s]: [p, d, s] = X.T[d*128+p, s]
            m["xt8"] = _to_f8(xt.reshape(nd, 128, s).transpose(1, 0, 2))
            for nm, W in (("wq", Wq), ("wk", Wk)):
                wt = np.ascontiguousarray(W[perm].T)  # [D, GW]
                # [128, nd//2, 2, GW]: [p, dd, i, c] = wt[(2dd+i)*128+p, c]
                m[nm] = _to_f8(
                    wt.reshape(nd // 2, 2, 128, GW).transpose(2, 0, 1, 3)
                )
        else:
            m["wq"] = np.ascontiguousarray(Wq[perm].T).astype(np.float16)
            m["wk"] = np.ascontiguousarray(Wk[perm].T).astype(np.float16)
        in_maps.append(m)
    return in_maps


def kernel(X, Wq, bq, Wk, bk, Wv, bv, Wo, bo, **run_kwargs):
    args = [np.asarray(a, np.float32) for a in (X, Wq, bq, Wk, bk, Wv, bv, Wo, bo)]
    X, Wq, bq, Wk, bk, Wv, bv, Wo, bo = args
    nc = _get_nc(S)
    in_maps = make_inputs(X, Wq, bq, Wk, bk, Wv, bv, Wo, bo, S, fp8_proj=FP8_PROJ)
    res = run_bass_kernel_spmd(nc, in_maps, core_ids=list(range(8)), **run_kwargs)
    outs = [r["out"] for r in res.results]
    full = np.empty((B, S, D), np.float32)
    for b in range(B):
        full[b] = outs[2 * b].astype(np.float32) + outs[2 * b + 1].astype(np.float32) + bo
    kernel.last_results = res
    return full


# revision 69
# speedup vs baseline: 1.0058x; 1.0021x over previous
"""Multi-head causal attention (B=4, S=2048, D=1024, H=16) on 8 NeuronCores.

Sharding: core c handles batch b = c//2 and head-group g = c%2 (8 heads).
Each core computes QKV projections for its group, causal attention for its
8 heads, and a partial output projection (row-split Wo).  Host sums the two
fp16 partials per batch in fp32 and adds bo.

On-chip design (per core):
  XT [D, S] = X[b].T fp16 in 8 chunks [128, S].
  Q/K are projected (stationary W-chunk x moving XT) and evacuated from
  PSUM straight to fp8e4 tiles qt8/kt8 [128, 2, S] (Q with a fused
  bias-add; K takes none - softmax is invariant to the key bias, so K
  evacs are plain casts and the prologue's run on the then-idle ScalarE):
  partition p of tile-group tg holds head 4*tg + p//32, dim1 selects the
  hd half (hd = dim1*32 + p%32).  Host permutes Wq/Wk columns to produce
  this layout directly.
  Scores run as fp8 DoubleRow matmuls (2 k-tiles of 32 partitions), giving
  qk [128 k, 2 heads, 512 q] PSUM blocks at half the fp16 cycle cost; exp
  on ScalarE (scale=1/8, no max subtraction; |scores| <= ~3) into fp16 et
  tiles (tag sets alternate by pair parity so the next pair's exps overlap
  the previous pair's PV reads; window 0 gets one set per pair).  Causal
  masking of diagonal 128x128 blocks happens inside the scores PSUM
  accumulation: an identity-stationary matmul adds 0/-30000 so exp zeroes
  the upper triangle with no extra vector pass.
  PV is TRANSPOSED: po [q=128, 65] += et_t[:, hh, qcol].T @ v_t[:, h, 0:65]
  accumulated over t (col 64 of v is ones and yields the exp rowsum), so
  the softmax divisor lands in a PSUM column: DVE reciprocal of po[:,64]
  then a per-partition tensor_scalar multiply evacuates the normalized
  [q, 64] tile in one shot (no scatter/gather/broadcast DMAs).  PV blocks
  emit all chains+norms first and the PE transposes (identity matmul,
  fp16 PSUM, two heads via tile_position col 0/64, two q-chunks per tile)
  afterwards, keeping DVE round-trip waits off the PE queue.
  O-projection runs at st-chunk granularity, evacuated fp32->fp16 on DVE
  (ScalarE Copy for the tail chunks), stored via per-half [128, 512] fp16
  DMAs; each window's O-projection is deferred two windows so the PE work
  lands in ACT-bound stretches.
  Emission is j-window-outer / pair-inner with V-projection chunks and
  Q/K projection groups interleaved as PE fillers, window 1's first score
  blocks pulled into window 0's tail, and each next window's first score
  block pulled into the previous weave, so ScalarE stays fed.

Walrus wait-slot legality (1 sem wait per engine instruction): Tile's
extra waits are split onto same-engine NoOps by _legalize_waits.
"""

import sys

for _p in ("/opt/trn_rl_repo",):
    if _p not in sys.path:
        sys.path.insert(0, _p)

from contextlib import ExitStack

import numpy as np

import concourse.bass as bass
import concourse.mybir as mybir
import concourse.tile as tile
from concourse.bass_utils import run_bass_kernel_spmd

import bass_rust

F16 = mybir.dt.float16
F32 = mybir.dt.float32
F8 = mybir.dt.float8e4
AF = mybir.ActivationFunctionType
DR = mybir.MatmulPerfMode.DoubleRow

B, S, D, H = 4, 2048, 1024, 16
HD = D // H  # 64
GH = 8  # heads per group
GW = GH * HD  # 512 columns per group


_SPLITTABLE = {
    "InstMatmult", "InstLdweights", "InstActivation", "InstTensorCopy",
    "InstTensorTensor", "InstTensorScalarPtr", "InstTensorReduce",
    "InstMemset", "InstDMACopy", "InstReciprocal", "InstIota",
    "InstTensorTensorReduce", "InstBNStats", "InstBNStatsAggregate",
    "InstStreamShuffle", "InstNoOp", "InstPool", "InstMax", "InstDrain",
}


def _legalize_waits(nc, max_waits=1):
    """Walrus codegen accepts at most one sync-wait command per engine
    instruction; Tile's wait assigner can emit more.  Split extras onto
    same-engine NoOps inserted immediately before (semantics preserved:
    the engine blocks at the same program point)."""
    ctr = 0
    for fn in nc.m.functions:
        for blk in fn.blocks:
            out = []
            for ins in blk.instructions:
                si = ins.sync_info
                if (
                    si is not None
                    and len(si.on_wait) > max_waits
                    and type(ins).__name__ in _SPLITTABLE
                ):
                    waits = list(si.on_wait)
                    extra, keep = waits[:-max_waits], waits[-max_waits:]
                    for w in extra:
                        nop = mybir.InstNoOp(name=f"waitnop-{ctr}", ins=[], outs=[])
                        ctr += 1
                        nop.engine = ins.engine
                        nop.sync_info = bass_rust.SyncInfo(on_wait=[w], on_update=[])
                        out.append(nop)
                    ins.sync_info = bass_rust.SyncInfo(
                        on_wait=keep, on_update=list(si.on_update)
                    )
                out.append(ins)
            blk.instructions[:] = out
    return ctr


def build_nc(s=S, fp8_proj=False, legalize=True):
    ns = s // 512  # 512-wide q windows
    nt = s // 128  # 128-wide s chunks
    nd = D // 128  # contraction chunks for projections

    nc = bass.Bass("TRN2", target_bir_lowering=False, debug=False)
    xt_d = nc.dram_tensor("xt", [D, s], F16, kind="ExternalInput").ap()
    if fp8_proj:
        xt8_d = nc.dram_tensor("xt8", [128, nd, s], F8, kind="ExternalInput").ap()
        wq_d = nc.dram_tensor("wq", [128, nd // 2, 2, GW], F8, kind="ExternalInput").ap()
        wk_d = nc.dram_tensor("wk", [128, nd // 2, 2, GW], F8, kind="ExternalInput").ap()
    else:
        wq_d = nc.dram_tensor("wq", [D, GW], F16, kind="ExternalInput").ap()
        wk_d = nc.dram_tensor("wk", [D, GW], F16, kind="ExternalInput").ap()
    wv_d = nc.dram_tensor("wv", [D, GW], F16, kind="ExternalInput").ap()
    wo_d = nc.dram_tensor("wo", [GW, D], F16, kind="ExternalInput").ap()
    bqk_d = nc.dram_tensor("bqk", [128, 8], F32, kind="ExternalInput").ap()
    mask_d = nc.dram_tensor("mask", [128, 128], F16, kind="ExternalInput").ap()
    iden_d = nc.dram_tensor("iden", [128, 128], F16, kind="ExternalInput").ap()
    out_d = nc.dram_tensor("out", [s, D], F16, kind="ExternalOutput").ap()

    with tile.TileContext(nc) as tc, ExitStack() as ctx:
        pool = lambda name, bufs, **kw: ctx.enter_context(
            tc.tile_pool(name=name, bufs=bufs, **kw)
        )
        const_p = pool("const", 1)
        xt_p = pool("xtp", nd)
        w_p = pool("wp", 1)
        qkt_p = pool("qktp", 1)
        v_p = pool("vp", nt)
        et_p = pool("etp", 1)  # tags e{parity}_{t}, one buf each
        ot_p = pool("otp", 4)
        on_p = pool("onp", 3)
        rec_p = pool("recp", 3)
        osb_p = pool("osbp", 3)
        ps_qk = pool("psqk", 2, space="PSUM")      # [128,2,512] f32 -> 4 banks
        ps_main = pool("psmain", 3, space="PSUM")  # [128,512] f32 -> 3 banks
        ps_tr = pool("pstr", 1, space="PSUM")      # [128,128] f16 -> 1 bank

        # --- input DMAs, in order of first use ---
        bqk_sb = const_p.tile([128, 8], F32)
        nc.sync.dma_start(out=bqk_sb[:], in_=bqk_d[:])
        if fp8_proj:
            xt8_sb = w_p.tile([128, nd, s], F8, name="xt8")
            nc.sync.dma_start(out=xt8_sb[:, 0:2, :], in_=xt8_d[:, 0:2, :])
            wq_sb = w_p.tile([128, nd // 2, 2, GW], F8, name="wq8")
            wk_sb = w_p.tile([128, nd // 2, 2, GW], F8, name="wk8")
            nc.sync.dma_start(out=wq_sb[:], in_=wq_d[:])
            nc.sync.dma_start(out=wk_sb[:], in_=wk_d[:])
            for dd in range(1, nd // 2):
                nc.sync.dma_start(
                    out=xt8_sb[:, 2 * dd : 2 * dd + 2, :],
                    in_=xt8_d[:, 2 * dd : 2 * dd + 2, :],
                )
            mask_sb = const_p.tile([128, 128], F16)
            nc.sync.dma_start(out=mask_sb[:], in_=mask_d[:])
            iden_sb = const_p.tile([128, 128], F16)
            nc.sync.dma_start(out=iden_sb[:], in_=iden_d[:])
            wv_sb = w_p.tile([128, nd, GW], F16, name="wv")
            nc.sync.dma_start(out=wv_sb[:], in_=wv_d.rearrange("(d p) n -> p d n", p=128))
            xt_sb = []
            for d in range(nd):
                t = xt_p.tile([128, s], F16, tag="xt", name=f"xtc{d}")
                nc.sync.dma_start(out=t[:], in_=xt_d[d * 128 : (d + 1) * 128, :])
                xt_sb.append(t)
        else:
            wq_sb = w_p.tile([128, nd, GW], F16, name="wq")
            nc.sync.dma_start(out=wq_sb[:], in_=wq_d.rearrange("(d p) n -> p d n", p=128))
            xt_sb = []
            for d in range(nd):
                t = xt_p.tile([128, s], F16, tag="xt", name=f"xtc{d}")
                nc.sync.dma_start(out=t[:], in_=xt_d[d * 128 : (d + 1) * 128, :])
                xt_sb.append(t)
            wk_sb = w_p.tile([128, nd, GW], F16, name="wk")
            nc.sync.dma_start(out=wk_sb[:], in_=wk_d.rearrange("(d p) n -> p d n", p=128))
            mask_sb = const_p.tile([128, 128], F16)
            nc.sync.dma_start(out=mask_sb[:], in_=mask_d[:])
            wv_sb = w_p.tile([128, nd, GW], F16, name="wv")
            nc.sync.dma_start(out=wv_sb[:], in_=wv_d.rearrange("(d p) n -> p d n", p=128))
        if not fp8_proj:
            iden_sb = const_p.tile([128, 128], F16)
            nc.sync.dma_start(out=iden_sb[:], in_=iden_d[:])
        wo_sb = w_p.tile([128, 4, D], F16, name="wo")
        nc.sync.dma_start(out=wo_sb[:], in_=wo_d.rearrange("(c p) n -> p c n", p=128))

        # touch ops: early Exp ACT-table load + const observations
        scr_a = const_p.tile([128, 1], F32)
        nc.scalar.activation(scr_a[:], bqk_sb[:, 0:1], AF.Exp, scale=1.0)
        scr_v = const_p.tile([128, 1], F16)
        nc.vector.tensor_copy(scr_v[:], mask_sb[:, 0:1])
        scr_m = const_p.tile([128, 1], F16)
        nc.vector.tensor_copy(scr_m[:], mask_sb[:, 0:1])
        scr_i = const_p.tile([128, 1], F16)
        nc.gpsimd.tensor_copy(scr_i[:], iden_sb[:, 0:1])

        ot_sb = [ot_p.tile([128, s], F16, tag="ot", name=f"ot{m}") for m in range(4)]
        qt8 = [qkt_p.tile([128, 2, s], F8, name=f"qt8_{tg}") for tg in range(2)]
        kt8 = [qkt_p.tile([128, 2, s], F8, name=f"kt8_{tg}") for tg in range(2)]

        # --- projection-group emitters (used as PE fillers) ---
        def qk_group(pi, tg, inst, sl, act_evac=False):
            """One Q/K projection PSUM group -> fp8 evacuation.  Q gets the
            bias folded in; K needs none (softmax is invariant to the key
            bias: the q.bk term is constant per query row and cancels in
            normalization), which also lets prologue K evacs run on the
            then-idle ScalarE."""
            dst = (qt8, kt8)[pi][tg]
            col = (tg * 2 + inst) * 128
            ps = ps_main.tile([128, 512], F32, tag="ps", name="ps")
            if fp8_proj:
                for dd in range(nd // 2):
                    nc.tensor.matmul(
                        ps[:],
                        wq_sb[:, dd, :, col : col + 128] if pi == 0
                        else wk_sb[:, dd, :, col : col + 128],
                        xt8_sb[:, 2 * dd : 2 * dd + 2, sl * 512 : (sl + 1) * 512],
                        start=(dd == 0),
                        stop=(dd == nd // 2 - 1),
                        perf_mode=DR,
                    )
            else:
                for d in range(nd):
                    nc.tensor.matmul(
                        ps[:],
                        (wq_sb, wk_sb)[pi][:, d, col : col + 128],
                        xt_sb[d][:, sl * 512 : (sl + 1) * 512],
                        start=(d == 0),
                        stop=(d == nd - 1),
                    )
            dslice = dst[:, inst, sl * 512 : (sl + 1) * 512]
            if pi == 1:
                if act_evac:
                    nc.scalar.activation(dslice, ps[:], AF.Copy)
                else:
                    nc.vector.tensor_copy(dslice, ps[:])
            else:
                bc = tg * 2 + inst
                nc.vector.tensor_scalar_add(dslice, ps[:], bqk_sb[:, bc : bc + 1])

        v_sb = [None] * nt

        def v_group_multi(sts):
            """d-major interleave across <=3 chunks so only the last
            d-matmul of each chain waits the final xt DMA."""
            pss = []
            for st in sts:
                pss.append(ps_main.tile([128, 512], F32, tag="ps", name="ps"))
            for d in range(nd):
                for ps, st in zip(pss, sts):
                    nc.tensor.matmul(
                        ps[:],
                        xt_sb[d][:, st * 128 : (st + 1) * 128],
                        wv_sb[:, d, :],
                        start=(d == 0),
                        stop=(d == nd - 1),
                    )
            for ps, st in zip(pss, sts):
                vt = v_p.tile([128, GH, 65], F16, tag="v", name=f"v{st}")
                nc.vector.memset(vt[:, :, 64:65], 1.0)
                nc.scalar.activation(
                    vt[:, :, 0:64],
                    ps[:].rearrange("p (h e) -> p h e", h=GH),
                    AF.Copy,
                )
                v_sb[st] = vt

        def v_group(st):
            """One V-projection s-chunk [128, 8, 65] with ones col."""
            ps = ps_main.tile([128, 512], F32, tag="ps", name="ps")
            for d in range(nd):
                nc.tensor.matmul(
                    ps[:],
                    xt_sb[d][:, st * 128 : (st + 1) * 128],
                    wv_sb[:, d, :],
                    start=(d == 0),
                    stop=(d == nd - 1),
                )
            vt = v_p.tile([128, GH, 65], F16, tag="v", name=f"v{st}")
            nc.vector.memset(vt[:, :, 64:65], 1.0)
            if st == 3:  # window 0: ScalarE is idle during the xt wait
                nc.scalar.activation(
                    vt[:, :, 0:64],
                    ps[:].rearrange("p (h e) -> p h e", h=GH),
                    AF.Copy,
                )
            else:
                nc.vector.tensor_copy(
                    vt[:, :, 0:64], ps[:].rearrange("p (h e) -> p h e", h=GH)
                )
            v_sb[st] = vt

        # --- attention building blocks ---
        et_tiles = [[None] * nt for _ in range(4)]  # by pair parity (j0: per pair)

        def scores_t(m, j, t):
            tg, i0 = m // 2, 2 * (m % 2)
            par = m if j == 0 else m % 2
            diag = t >= 4 * j
            w0 = 128 * (t - 4 * j) if diag else 0
            qk = ps_qk.tile([128, 2, 512], F32, tag="qk", name="qk")
            for hh in range(2):
                ib = 32 * (i0 + hh)
                nc.tensor.matmul(
                    qk[:, hh, w0:512],
                    kt8[tg][ib : ib + 32, :, t * 128 : (t + 1) * 128],
                    qt8[tg][ib : ib + 32, :, j * 512 + w0 : (j + 1) * 512],
                    start=True,
                    stop=not diag,
                    perf_mode=DR,
                    tile_position=(ib, 0),
                    skip_group_check=diag,
                )
                if diag:
                    # causal mask: accumulate 0/-30000 into the diagonal
                    # 128x128 block (identity-stationary matmul) so exp
                    # zeroes the upper triangle with no DVE pass
                    nc.tensor.matmul(
                        qk[:, hh, w0 : w0 + 128],
                        iden_sb[:],
                        mask_sb[:],
                        start=False,
                        stop=True,
                        skip_group_check=True,
                    )
            et = et_p.tile([128, 2, 512], F16, tag=f"e{par}_{t}", name=f"e{par}_{t}")
            nc.scalar.activation(
                et[:, :, w0:512], qk[:, :, w0:512], AF.Exp, scale=0.125
            )
            et_tiles[par][t] = et

        def scores_block(m, j):
            for t in range(4 * j + 4):
                scores_t(m, j, t)

        trp_cur = {}

        def pv_chains(m, j, qq):
            """PV chains + norms only (no PE-blocking transpose waits)."""
            tg, i0 = m // 2, 2 * (m % 2)
            par = m if j == 0 else m % 2
            h0, h1 = 4 * tg + i0, 4 * tg + i0 + 1
            qc = 4 * j + qq
            pair = []
            for hh, h in ((0, h0), (1, h1)):
                po = ps_main.tile([128, 512], F32, tag="ps", name="po")
                for t in range(qc + 1):
                    nc.tensor.matmul(
                        po[:, 0:65],
                        et_tiles[par][t][:, hh, qq * 128 : (qq + 1) * 128],
                        v_sb[t][:, h, 0:65],
                        start=(t == 0),
                        stop=(t == qc),
                    )
                rec = rec_p.tile([128, 1], F32, tag="rec", name="rec")
                nc.vector.reciprocal(rec[:], po[:, 64:65])
                on = on_p.tile(
                    [128, 64], F16, tag=f"dn{m % 2}_{qq}_{hh}", name="dn", bufs=1
                )
                nc.vector.tensor_scalar_mul(on[:], po[:, 0:64], rec[:, 0:1])
                pair.append(on)
            return pair

        def pv_flush(m, j, ons, qq0=0):
            for qp in range(len(ons) // 2):
                trp = ps_tr.tile([128, 2, 128], F16, tag="tr", name="trp")
                for sub in range(2):
                    for hh in range(2):
                        nc.tensor.transpose(
                            trp[64 * hh : 64 * hh + 64, sub, :],
                            ons[2 * qp + sub][hh][:],
                            iden_sb[:],
                            tile_position=(0, 64 * hh),
                        )
                qc = 4 * j + qq0 + 2 * qp + 1
                nc.vector.tensor_copy(
                    ot_sb[m][:, (qc - 1) * 128 : (qc + 1) * 128],
                    trp[:].rearrange("p a b -> p (a b)"),
                )

        def pv_chunk_defer(m, j):
            ons = [pv_chains(m, j, qq) for qq in range(4)]
            return lambda: pv_flush(m, j, ons)

        def pv_chunk(m, j, qq):
            tg, i0 = m // 2, 2 * (m % 2)
            par = m if j == 0 else m % 2
            h0, h1 = 4 * tg + i0, 4 * tg + i0 + 1
            qc = 4 * j + qq
            # transpose targets pair up (two qc's per [128, 256] fp16 PSUM
            # tile) so the DVE evacuation is one copy per pair; the last
            # pair's O-proj chunks stay per-qc for the tail
            solo = m == 3 and j == ns - 1 and qq >= 2  # tail chunks stream per-qc
            if qq % 2 == 0 or solo:
                trp_cur[m] = ps_tr.tile([128, 2, 128], F16, tag="tr", name="trp")
            trp = trp_cur[m]
            for hh, h in ((0, h0), (1, h1)):
                po = ps_main.tile([128, 512], F32, tag="ps", name="po")
                for t in range(qc + 1):
                    nc.tensor.matmul(
                        po[:, 0:65],
                        et_tiles[par][t][:, hh, qq * 128 : (qq + 1) * 128],
                        v_sb[t][:, h, 0:65],
                        start=(t == 0),
                        stop=(t == qc),
                    )
                rec = rec_p.tile([128, 1], F32, tag="rec", name="rec")
                nc.vector.reciprocal(rec[:], po[:, 64:65])
                on = on_p.tile([128, 64], F16, tag="on", name="on")
                nc.vector.tensor_scalar_mul(on[:], po[:, 0:64], rec[:, 0:1])
                nc.tensor.transpose(
                    trp[64 * hh : 64 * hh + 64, 0 if solo else qq % 2, :],
                    on[:],
                    iden_sb[:],
                    tile_position=(0, 64 * hh),
                )
            if solo:
                nc.vector.tensor_copy(
                    ot_sb[m][:, qc * 128 : (qc + 1) * 128], trp[:, 0, :]
                )
                oproj_chunk(j, qq, act_evac=True)
            elif qq % 2 == 1:
                nc.vector.tensor_copy(
                    ot_sb[m][:, (qc - 1) * 128 : (qc + 1) * 128],
                    trp[:].rearrange("p a b -> p (a b)"),
                )
                if m == 3 and j == ns - 1:
                    oproj_chunk(j, qq - 1)
                    oproj_chunk(j, qq)

        def pv_block(m, j):
            pv_chunk_defer(m, j)()

        def scores_pv_block(m, j, mid=None, post=None):
            """Scores with the PV chunks woven between the diagonal t's so
            the window tail overlaps PE work with the last exps.  `post`
            (next window's first scores) lands after the last woven one."""
            for t in range(4 * j + 1):
                scores_t(m, j, t)
            if mid is not None:
                mid()
            if j == ns - 1:
                # tail: chains inline, pair-flush + oproj after the next
                # woven score so no DVE transpose-wait precedes an exp
                ons = []
                for qq in range(4):
                    if qq < 3:
                        scores_t(m, j, 4 * j + 1 + qq)
                    ons.append(pv_chains(m, j, qq))
                    if qq == 2 and post is not None:
                        post()
                    if qq % 2 == 1:
                        pv_flush(m, j, ons[qq - 1 : qq + 1], qq0=qq - 1)
                        oproj_chunk(j, qq - 1, act_evac=qq == 3)
                        oproj_chunk(j, qq, act_evac=qq == 3)
            else:
                ons = []
                for qq in range(4):
                    if qq < 3:
                        scores_t(m, j, 4 * j + 1 + qq)
                    ons.append(pv_chains(m, j, qq))
                    if qq == 2 and post is not None:
                        post()
                pv_flush(m, j, ons)

        def oproj_chunk(j, qq, act_evac=False):
            st = 4 * j + qq
            for dsl in range(2):
                po2 = ps_main.tile([128, 512], F32, tag="ps", name="po2")
                for cc in range(4):
                    nc.tensor.matmul(
                        po2[:],
                        ot_sb[cc][:, st * 128 : (st + 1) * 128],
                        wo_sb[:, cc, dsl * 512 : (dsl + 1) * 512],
                        start=(cc == 0),
                        stop=(cc == 3),
                    )
                osb = osb_p.tile([128, 512], F16, tag="osb", name="osb")
                if act_evac and dsl == 1:
                    # post-exp tail: ScalarE is idle, split the serial
                    # evacuation chain across both engines
                    nc.scalar.activation(osb[:], po2[:], AF.Copy)
                else:
                    nc.vector.tensor_copy(osb[:], po2[:])
                nc.sync.dma_start(
                    out=out_d[
                        st * 128 : (st + 1) * 128, dsl * 512 : (dsl + 1) * 512
                    ],
                    in_=osb[:],
                )

        # --- emission: prologue + j-outer / pair-inner windows ---
        # Projection-group units: (tg, sl) -> 4 groups (Q i0/i1, K i0/i1).
        # Emitted lazily as PE fillers; ensure() forces a unit's deadline.
        unit_q = [(0, 0)] + [
            (tg, sl) for sl in range(ns) for tg in range(2) if (tg, sl) != (0, 0)
        ]
        done_units = set()

        def emit_unit(tg, sl, act_evac=False):
            for pi in range(2):
                for inst in range(2):
                    qk_group(pi, tg, inst, sl, act_evac=act_evac)
            done_units.add((tg, sl))

        def ensure(tg, sl):
            while (tg, sl) not in done_units:
                fill_unit()

        def fill_unit():
            for u in unit_q:
                if u not in done_units:
                    emit_unit(*u)
                    return

        emit_unit(0, 0, act_evac=True)  # prologue: K evacs on idle ScalarE

        def oproj_block(j):
            for qq in range(4):
                oproj_chunk(j, qq)

        # Window 0: V (fp16 xt) lands late in the DMA stream and the window
        # has little exp work, so all four pairs' scores are front-loaded,
        # V chunks woven into pair 3's diagonal, and window 1's first two
        # score blocks pulled in to cover the PV tail.  O-projections are
        # deferred two windows (PE work moves into the ACT-bound windows).
        scores_block(0, 0)
        scores_block(1, 0)
        ensure(1, 0)
        if ns > 1:
            ensure(0, 1)
        fill_unit()  # (1,1): unblocks window 1 while PE/DVE are idle
        scores_block(2, 0)
        scores_t(3, 0, 0)
        scores_t(3, 0, 1)
        scores_t(3, 0, 2)
        scores_t(3, 0, 3)
        v_group_multi([0, 1, 2])
        v_group(3)
        if ns > 1:
            flush0 = pv_chunk_defer(0, 0)
            scores_block(0, 1)
            flush0()
        else:
            pv_block(0, 0)
        pv_block(1, 0)
        if ns > 1:
            scores_block(1, 1)
            pv_block(2, 0)
            v_group(4)
            v_group(5)
        else:
            pv_block(2, 0)
            for qq in range(4):
                pv_chunk(3, 0, qq)

        for j in range(1, ns):
            if j > 1:
                scores_block(1, j)  # scores_block(0, j) was pulled into
                v_group(4 * j)      # the previous window's weave
                v_group(4 * j + 1)
            ensure(1, j)
            v_group(4 * j + 2)
            v_group(4 * j + 3)
            pv_block(0, j)
            if j >= 2:
                oproj_block(j - 2)
            scores_block(2, j)
            if j == ns - 1 and j >= 1:
                oproj_block(j - 1)
            pv_block(1, j)
            if j == 1:
                pv_block(3, 0)
            last = j == ns - 1

            def mid(j=j):
                pv_block(2, j)

            def post(j=j, last=last):
                if not last:
                    ensure(0, j + 1)
                    scores_block(0, j + 1)

            scores_pv_block(3, j, mid=mid, post=post)

    if legalize:
        _legalize_waits(nc)
    return nc


_NC_CACHE = {}
FP8_PROJ = True


def _get_nc(s=S):
    key = (s, FP8_PROJ)
    if key not in _NC_CACHE:
        _NC_CACHE[key] = build_nc(s, fp8_proj=FP8_PROJ)
    return _NC_CACHE[key]


def _perm(lo):
    """Column permutation for Wq/Wk: new col a = tg*256 + inst*128 + p maps
    to original output dim lo + 64*(4*tg + p//32) + inst*32 + p%32."""
    a = np.arange(GW)
    tg, r = a // 256, a % 256
    inst, p = r // 128, r % 128
    return lo + 64 * (4 * tg + p // 32) + inst * 32 + (p % 32)


def _to_f8(a):
    import ml_dtypes

    return a.astype(ml_dtypes.float8_e4m3)


def make_inputs(X, Wq, bq, Wk, bk, Wv, bv, Wo, bo, s=S, fp8_proj=False):
    """Per-core input maps. Core c: batch c//2, head group c%2."""
    iv, jv = np.arange(128)[:, None], np.arange(128)[None, :]
    mask = np.where(jv >= iv, 0.0, -30000.0).astype(np.float16)
    iden = np.eye(128, dtype=np.float16)
    nd = D // 128
    in_maps = []
    for c in range(8):
        b, g = divmod(c, 2)
        lo, hi = g * GW, (g + 1) * GW
        perm = _perm(lo)
        bqk = np.empty((128, 8), np.float32)
        for pi, bias in enumerate((bq, bk)):
            for tg in range(2):
                for inst in range(2):
                    cols = perm[tg * 256 + inst * 128 : tg * 256 + inst * 128 + 128]
                    bqk[:, pi * 4 + tg * 2 + inst] = bias[cols]
        xt = np.ascontiguousarray(X[b, :s].T).astype(np.float16)
        m = {
            "xt": xt,
            "wv": np.ascontiguousarray(Wv[lo:hi].T).astype(np.float16),
            "wo": np.ascontiguousarray(Wo[:, lo:hi].T).astype(np.float16),
            "bqk": bqk,
            "mask": mask,
            "iden": iden,
        }
        if fp8_proj:
            # xt8 [128, nd, s]: [p, d, s] = X.T[d*128+p, s]
            m["xt8"] = _to_f8(xt.reshape(nd, 128, s).transpose(1, 0, 2))
            for nm, W in (("wq", Wq), ("wk", Wk)):
                wt = np.ascontiguousarray(W[perm].T)  # [D, GW]
                # [128, nd//2, 2, GW]: [p, dd, i, c] = wt[(2dd+i)*128+p, c]
                m[nm] = _to_f8(
                    wt.reshape(nd // 2, 2, 128, GW).transpose(2, 0, 1, 3)
                )
        else:
            m["wq"] = np.ascontiguousarray(Wq[perm].T).astype(np.float16)
            m["wk"] = np.ascontiguousarray(Wk[perm].T).astype(np.float16)
        in_maps.append(m)
    return in_maps


def kernel(X, Wq, bq, Wk, bk, Wv, bv, Wo, bo, **run_kwargs):
    args = [np.asarray(a, np.float32) for a in (X, Wq, bq, Wk, bk, Wv, bv, Wo, bo)]
    X, Wq, bq, Wk, bk, Wv, bv, Wo, bo = args
    nc = _get_nc(S)
    in_maps = make_inputs(X, Wq, bq, Wk, bk, Wv, bv, Wo, bo, S, fp8_proj=FP8_PROJ)
    res = run_bass_kernel_spmd(nc, in_maps, core_ids=list(range(8)), **run_kwargs)
    outs = [r["out"] for r in res.results]
    # V-bias is softmax-transparent: it adds bv to the attention output,
    # which the O-projection maps to the constant Wo @ bv -> fold into bo
    bo_eff = bo + Wo @ bv
    full = np.empty((B, S, D), np.float32)
    for b in range(B):
        full[b] = (
            outs[2 * b].astype(np.float32)
            + outs[2 * b + 1].astype(np.float32)
            + bo_eff
        )
    kernel.last_results = res
    return full
